# revision 26
# baseline (speedup 1.0000x reference)
"""Chamfer loss kernel for Trainium2 (8 NeuronCores, one batch per core).

Problem: B=8, N=M=8192, D=64 fp32.
  rd = pairwise euclidean distances x[b] vs y[b]   [B, N, M]
  loss = mean_b( sum_n min_m rd + sum_m min_n rd ) / M

Device strategy (per core = one batch):
  - sqrt is monotonic -> only need minima of SQUARED distances; sqrt+sums
    happen on host over 2*8192 values per batch.
  - d2 = x2 + y2 - 2*x.y is produced entirely by ONE bf16 matmul with an
    augmented contraction dim:
       lhsT rows (x side, [68, N]): [x_d (64) ; 1 ; 1 ; x2_hi ; x2_lo]
       rhs  rows (y side, [68, M]): [-2*y_d (64) ; y2_hi ; y2_lo ; 1 ; 1]
    so psum = sum_d x_d*(-2 y_d) + y2_hi + y2_lo + x2_hi + x2_lo = d2.
    (hi/lo bf16 splits keep the squared-norm terms at ~fp24 precision.)
  - ScalarE copies each PSUM group into bf16 SBUF tiles; VectorE (the
    bottleneck engine, bf16 tensor_tensor min at 2 elem/cycle/lane) does
    all min work. The default "fold9" schedule minimizes DVE instruction
    COUNT at constant 2x busy work (each DVE instruction was measured to
    carry ~0.5-1us of hidden cost beyond its streaming time):
      * sfull tiles land in [P, 2, n] PAIR stacks; the column accumulator
        is a [P, 2, n] tile (even tiles fold into slot 0, odd into slot 1)
        updated by ONE flat 16K-wide 2x TT per pair, merged once at the end.
      * per-tile row fold tree of 2D contiguous TT-mins down to 512, the
        last fold writing a [P, 8, 512] stack; ONE batched 1x tensor_reduce
        per 8 tiles. (Strided/3D TTs demote to 1x on HW - never use them;
        small per-tile reduces cost ~1us hidden each - batch them.)
  - The col accumulator is finished by PE transposes + wide DVE reduces.
Host does the final sqrt / sums / mean in float64.
Measured (repeat-module min-stats): fold 566us -> fold4 504us -> fold9
480us. Rejected by experiment: tensor_tensor_reduce (crashes the runtime),
vector.max top-8 (6.8us/tile > tree), gpsimd tensor_tensor min / DMA-CCE
min (unsupported by the compiler), moving copies to DVE (fp32-PSUM copy
is ~4us/op there), single-buffer paired PSUM copies (PE p-state throttle).
"""

import os

import numpy as np
import ml_dtypes

P = 128
N = 8192
D = 64
KAUG = D + 4  # 68
B = 8

_CACHE = {}

DEFAULT_ROW_MODE = "fold9"


SOFT_T = float(os.environ.get("CHAMFER_SOFT_T", "2.0"))
SOFT_REF = float(os.environ.get("CHAMFER_SOFT_REF", "10.0"))
SOFT_W = int(os.environ.get("CHAMFER_SOFT_W", "4096"))   # soft m-width
WGP = int(os.environ.get("CHAMFER_WGP", "2048"))         # gpsimd col width
SOFT_ACCUM = int(os.environ.get("CHAMFER_SOFT_ACCUM", "1"))


def _build_nc(n=N, mm_free=512, m_group=2048, row_mode="ttr", kaug=KAUG,
              skip_tail=False, repeat=1, col_tail="device", dve_copies=0):
    import concourse.bass as bass
    import concourse.mybir as mybir
    import concourse.tile as tile
    from concourse import bacc
    from concourse.masks import make_identity

    fp32 = mybir.dt.float32
    bf16 = mybir.dt.bfloat16
    MIN = mybir.AluOpType.min

    nt_count = n // P          # n-tiles (output partition blocks)
    ngroups = n // m_group     # m groups per n-tile
    mm_per_g = m_group // mm_free

    # Bacc (not raw Bass): its compile pipeline lowers instructions with more
    # sync waits than the ISA's embedded slots into EventSemaphore insts.
    nc = bacc.Bacc("TRN2", target_bir_lowering=False, debug=False)
    xT = nc.dram_tensor("xT", [kaug, n], bf16, kind="ExternalInput")
    yT = nc.dram_tensor("yT", [kaug, n], bf16, kind="ExternalInput")
    rows_w = nt_count * 8 if row_mode == "max8" else nt_count
    if row_mode == "fold17":
        # [rowmin_e 64 | rsoft 64 | colmin_e 48 | colsum_soft 16]
        out = nc.dram_tensor("out", [P, 192], fp32, kind="ExternalOutput")
    else:
        out = nc.dram_tensor("out", [P, rows_w + nt_count], fp32,
                             kind="ExternalOutput")
    colout = None
    if col_tail == "host":
        # ship the lane-folded col accumulator; host does the 128-lane min
        colout = nc.dram_tensor("colout", [P, n], bf16, kind="ExternalOutput")

    with tile.TileContext(nc) as tc:
        with (
            tc.tile_pool(name="const", bufs=1) as cpool,
            tc.tile_pool(name="work", bufs=3) as wpool,
            tc.tile_pool(name="psum", bufs=2, space="PSUM") as ppool,
        ):
            xTs = cpool.tile([P, n], bf16)
            yTs = cpool.tile([P, n], bf16)
            colacc = (cpool.tile([P, n], bf16, name="colacc")
                      if row_mode not in ("fold9", "fold10", "fold11", "fold12", "fold17")
                      else None)
            rowacc = (cpool.tile([P, m_group], bf16, name="rowacc")
                      if row_mode in ("ttr", "ttr2", "tt") else None)
            rowmin = cpool.tile([P, nt_count], fp32)
            if col_tail != "host":
                colmin = cpool.tile([P, nt_count], fp32)
                ident = cpool.tile([P, P], bf16)

            # chunked loads so early matmuls start before the full tensors land
            n_chunks = max(1, n // 2048)
            cw = n // n_chunks
            for c in range(n_chunks):
                nc.sync.dma_start(
                    xTs[:kaug, c * cw:(c + 1) * cw], xT[:, c * cw:(c + 1) * cw])
                nc.sync.dma_start(
                    yTs[:kaug, c * cw:(c + 1) * cw], yT[:, c * cw:(c + 1) * cw])
            if col_tail != "host":
                make_identity(nc, ident)

            if row_mode == "tt":
                rowacc_narrow = cpool.tile([P, mm_free], bf16)
            if row_mode == "ttr2":
                rowacc2 = cpool.tile([P, m_group], bf16)

            if row_mode == "fold2":
                # alias-free variant of "fold": ping-pong col accumulators and
                # alternate row-tree scratch tiles, in case in-place operands
                # demote the DVE from 2x_1P to 1x mode.
                colacc2 = cpool.tile([P, n], bf16)
                accs = [colacc, colacc2]
                vtile = cpool.tile([P, n // 4], bf16)
                for rep in range(repeat):
                    for nt in range(nt_count):
                        lhsT = xTs[:kaug, nt * P:(nt + 1) * P]
                        sfull = wpool.tile([P, n], bf16, tag="s",
                                           name="sfull", bufs=3)
                        for g in range(ngroups):
                            ps = ppool.tile([P, m_group], fp32,
                                            tag="ps", name="ps")
                            for k in range(mm_per_g):
                                nc.tensor.matmul(
                                    ps[:, k * mm_free:(k + 1) * mm_free],
                                    lhsT,
                                    yTs[:kaug,
                                        g * m_group + k * mm_free:
                                        g * m_group + (k + 1) * mm_free],
                                    start=True, stop=True)
                            nc.scalar.copy(
                                out=sfull[:, g * m_group:(g + 1) * m_group],
                                in_=ps)
                        i = (rep * nt_count + nt) % 2
                        if nt == 0 and rep == 0:
                            nc.vector.tensor_copy(out=accs[i], in_=sfull)
                        else:
                            nc.vector.tensor_tensor(
                                out=accs[i], in0=accs[1 - i], in1=sfull,
                                op=MIN)
                        # row fold tree, alternating scratch tiles (no alias)
                        u = wpool.tile([P, n // 2], bf16, tag="u",
                                       name="u", bufs=3)
                        nc.vector.tensor_tensor(
                            out=u, in0=sfull[:, :n // 2],
                            in1=sfull[:, n // 2:], op=MIN)
                        nc.vector.tensor_tensor(
                            out=vtile, in0=u[:, :n // 4],
                            in1=u[:, n // 4:], op=MIN)
                        nc.vector.tensor_tensor(
                            out=u[:, :n // 8], in0=vtile[:, :n // 8],
                            in1=vtile[:, n // 8:], op=MIN)
                        nc.vector.tensor_tensor(
                            out=vtile[:, :n // 16], in0=u[:, :n // 16],
                            in1=u[:, n // 16:n // 8], op=MIN)
                        nc.vector.tensor_reduce(
                            out=rowmin[:, nt:nt + 1], in_=vtile[:, :n // 16],
                            axis=mybir.AxisListType.X, op=MIN)
                final_colacc = accs[(repeat * nt_count - 1) % 2]
            elif row_mode in ("fold9", "fold10", "fold11", "fold12", "fold17"):
                final_colacc = None  # set after the mode's loop builds it
            else:
                final_colacc = colacc

            if row_mode == "ttrh":
                # One n-wide s tile per n-tile: ONE wide col-min TT, and the
                # ENTIRE row reduction as ONE tensor_tensor_reduce: the
                # elementwise out is the min of the two m-halves (first fold
                # level) and accum_out is the running min over the free dim of
                # that out — i.e. the true row min. Saves the fold tree and
                # the 1x tensor_reduce of the "fold" mode.
                for rep in range(repeat):
                    for nt in range(nt_count):
                        lhsT = xTs[:kaug, nt * P:(nt + 1) * P]
                        sfull = wpool.tile([P, n], bf16, tag="s",
                                           name="sfull", bufs=3)
                        for g in range(ngroups):
                            ps = ppool.tile([P, m_group], fp32,
                                            tag="ps", name="ps")
                            for k in range(mm_per_g):
                                nc.tensor.matmul(
                                    ps[:, k * mm_free:(k + 1) * mm_free],
                                    lhsT,
                                    yTs[:kaug,
                                        g * m_group + k * mm_free:
                                        g * m_group + (k + 1) * mm_free],
                                    start=True, stop=True)
                            nc.scalar.copy(
                                out=sfull[:, g * m_group:(g + 1) * m_group],
                                in_=ps)
                        if nt == 0 and rep == 0:
                            nc.vector.tensor_copy(out=colacc, in_=sfull)
                        else:
                            nc.vector.tensor_tensor(
                                out=colacc, in0=colacc, in1=sfull, op=MIN)
                        u = wpool.tile([P, n // 2], bf16, tag="u",
                                       name="u", bufs=3)
                        nc.vector.tensor_tensor_reduce(
                            out=u,
                            in0=sfull[:, :n // 2],
                            in1=sfull[:, n // 2:],
                            scale=1.0,
                            scalar=3.0e38,
                            op0=MIN,
                            op1=MIN,
                            accum_out=rowmin[:, nt:nt + 1],
                        )

            if row_mode == "max8":
                # Inputs are NEGATED (-d2, see _prep_inputs(negate=True)) so
                # minima become maxima and the DVE's Max instruction (top-8
                # per partition in one op, free size up to 16K) computes each
                # n-tile's row extreme in ONE instruction, replacing the fold
                # tree + 1x tensor_reduce. Col side is a wide TT-max chain.
                # PSUM->SBUF copies are PAIRED (one 4096-wide ScalarE copy per
                # two m-groups) out of a single full-PSUM [P, 2, m_group]
                # tile: PE and ScalarE serialize on the 8 banks, but their
                # combined 7.4us/tile cadence stays under the DVE's ~9us.
                MAX = mybir.AluOpType.max
                rowtop = cpool.tile([P, nt_count, 8], fp32)
                for rep in range(repeat):
                    for nt in range(nt_count):
                        lhsT = xTs[:kaug, nt * P:(nt + 1) * P]
                        sfull = wpool.tile([P, n], bf16, tag="s",
                                           name="sfull", bufs=3)
                        for g in range(ngroups):
                            ps = ppool.tile([P, m_group], fp32,
                                            tag="ps", name="ps")
                            for k in range(mm_per_g):
                                nc.tensor.matmul(
                                    ps[:, k * mm_free:(k + 1) * mm_free],
                                    lhsT,
                                    yTs[:kaug,
                                        g * m_group + k * mm_free:
                                        g * m_group + (k + 1) * mm_free],
                                    start=True, stop=True)
                            dst = sfull[:, g * m_group:(g + 1) * m_group]
                            if g >= ngroups - dve_copies:
                                nc.vector.tensor_copy(out=dst, in_=ps)
                            else:
                                nc.scalar.copy(out=dst, in_=ps)
                        if nt == 0 and rep == 0:
                            nc.vector.tensor_copy(out=colacc, in_=sfull)
                        else:
                            nc.vector.tensor_tensor(
                                out=colacc, in0=colacc, in1=sfull, op=MAX)
                        nc.vector.max(rowtop[:, nt, :], sfull)

            if row_mode == "fold10":
                # fold9 + interleaved pair layout: the ScalarE copies place
                # the two tiles' m-groups interleaved in a flat [P, 2n] pair
                # buffer (block 2g+t = tile t, group g), so the first
                # log2(ngroups) row-fold levels are ALSO single flat 2x TTs
                # per PAIR (halves of the region pair elements of the same
                # tile). ~3.6 DVE instructions per tile. The column pair
                # accumulator keeps the interleaved layout and is merged into
                # natural m order by ngroups small TTs once at the end.
                KRA = 8
                wpt = m_group
                while wpt > 1024:
                    wpt //= 2
                cacc2 = cpool.tile([P, 2 * n], bf16)
                colaccX = cpool.tile([P, n], bf16)
                rstA = cpool.tile([P, KRA, wpt // 2], bf16)
                npair_folds = 0
                gg = ngroups
                while gg > 1:
                    gg //= 2
                    npair_folds += 1
                for rep in range(repeat):
                    for nt in range(nt_count):
                        lhsT = xTs[:kaug, nt * P:(nt + 1) * P]
                        t2 = nt % 2
                        if t2 == 0:
                            spairF = wpool.tile([P, 2 * n], bf16, tag="sp",
                                                name="spairF", bufs=2)
                        for g in range(ngroups):
                            ps = ppool.tile([P, m_group], fp32,
                                            tag="ps", name="ps")
                            for k in range(mm_per_g):
                                nc.tensor.matmul(
                                    ps[:, k * mm_free:(k + 1) * mm_free],
                                    lhsT,
                                    yTs[:kaug,
                                        g * m_group + k * mm_free:
                                        g * m_group + (k + 1) * mm_free],
                                    start=True, stop=True)
                            blk = 2 * g + t2
                            nc.scalar.copy(
                                out=spairF[:, blk * m_group:
                                           (blk + 1) * m_group],
                                in_=ps)
                        if t2 == 1:
                            if nt == 1 and rep == 0:
                                nc.vector.tensor_copy(out=cacc2, in_=spairF)
                            else:
                                nc.vector.tensor_tensor(
                                    out=cacc2, in0=cacc2, in1=spairF, op=MIN)
                            # paired fold levels (flat 2D, both tiles at once)
                            upair = wpool.tile([P, n], bf16, tag="u",
                                               name="upair", bufs=2)
                            nc.vector.tensor_tensor(
                                out=upair, in0=spairF[:, :n],
                                in1=spairF[:, n:], op=MIN)
                            w = n
                            for _ in range(npair_folds - 1):
                                nc.vector.tensor_tensor(
                                    out=upair[:, :w // 2],
                                    in0=upair[:, :w // 2],
                                    in1=upair[:, w // 2:w], op=MIN)
                                w //= 2
                            # now upair[:, :w] = [t0_block | t1_block], each
                            # m_group wide; per-tile folds + reduce stack
                            for tt in range(2):
                                base = tt * (w // 2)
                                ww = w // 2
                                while ww > 1024:
                                    nc.vector.tensor_tensor(
                                        out=upair[:, base:base + ww // 2],
                                        in0=upair[:, base:base + ww // 2],
                                        in1=upair[:, base + ww // 2:
                                                  base + ww], op=MIN)
                                    ww //= 2
                                ntt = nt - 1 + tt
                                nc.vector.tensor_tensor(
                                    out=rstA[:, ntt % KRA, :],
                                    in0=upair[:, base:base + ww // 2],
                                    in1=upair[:, base + ww // 2:base + ww],
                                    op=MIN)
                            if (nt % KRA) == KRA - 1:
                                nc.vector.tensor_reduce(
                                    out=rowmin[:, nt - KRA + 1:nt + 1],
                                    in_=rstA[:, :, :],
                                    axis=mybir.AxisListType.X, op=MIN)
                # merge interleaved col accumulator into natural m order
                for g in range(ngroups):
                    nc.vector.tensor_tensor(
                        out=colaccX[:, g * m_group:(g + 1) * m_group],
                        in0=cacc2[:, 2 * g * m_group:(2 * g + 1) * m_group],
                        in1=cacc2[:, (2 * g + 1) * m_group:
                                  (2 * g + 2) * m_group],
                        op=MIN)
                final_colacc = colaccX

            if row_mode == "fold17":
                # Soft/exact hybrid. m-groups g0..g2 (6144 cols) stay exact
                # bf16 d2: per-tile direct MIN tensor_reduce for rows (2x on
                # HW) + a paired 2x TT-MIN column chain. Group g3 (2048 cols)
                # is evacuated by ScalarE as exp((ref-d2)/T) (ACT free
                # affine; exp from PSUM measured ~2 elem/cycle) with
                # accum_out producing the row softmin sums for free, and its
                # COLUMN accumulation runs entirely on the otherwise-idle
                # GPSIMD engine as a paired bf16 TT-ADD chain (softmin).
                # Host recovers d2 = ref - T*ln(sum) and combines.
                EXP = mybir.ActivationFunctionType.Exp
                ADD = mybir.AluOpType.add
                MAX = mybir.AluOpType.max
                we = n - m_group          # exact width 6144
                wsoft = m_group           # soft width 2048
                # gpsimd Add measured ~5ns/elem: cap its slice so the single
                # Pool engine stays under the target cadence; the rest of the
                # soft columns fold on DVE as a 2x MAX chain (exact-in-exp).
                wgp = min(wsoft, (WGP // P) * P)
                wdm = wsoft - wgp         # DVE MAX-chain slice of g3
                cacc_e = cpool.tile([P, 2, we], bf16, name="cacc_e")
                gacc = (cpool.tile([P, 2, wgp], bf16, name="gacc")
                        if wgp else None)
                cacc_m = (cpool.tile([P, 2, wdm], bf16, name="cacc_m")
                          if wdm else None)
                rsoft = cpool.tile([P, nt_count], fp32, name="rsoft")
                colsum_s = cpool.tile([P, wsoft // P], fp32, name="colsum_s")
                KR = 8
                fw = we // 8  # 768: tree folds 6144 -> 768 into the stack
                rst = cpool.tile([P, KR, fw], bf16, name="rst17")
                if wgp:
                    nc.vector.memset(gacc[:, :, :], 0.0)
                if not SOFT_ACCUM:
                    nc.vector.memset(rsoft[:, :], 0.0)
                for rep in range(repeat):
                    for nt in range(nt_count):
                        lhsT = xTs[:kaug, nt * P:(nt + 1) * P]
                        s2 = nt % 2
                        if s2 == 0:
                            spair = wpool.tile([P, 2, we], bf16, tag="sp",
                                               name="spair", bufs=2)
                            epair = wpool.tile(
                                [P, 2, wsoft], bf16, tag="ep", name="epair",
                                bufs=int(os.environ.get("CHAMFER_EP_BUFS",
                                                        "3")))
                        for g in range(ngroups):
                            ps = ppool.tile([P, m_group], fp32,
                                            tag="ps", name="ps")
                            for k in range(mm_per_g):
                                nc.tensor.matmul(
                                    ps[:, k * mm_free:(k + 1) * mm_free],
                                    lhsT,
                                    yTs[:kaug,
                                        g * m_group + k * mm_free:
                                        g * m_group + (k + 1) * mm_free],
                                    start=True, stop=True)
                            if g < ngroups - 1:
                                nc.scalar.copy(
                                    out=spair[:, s2,
                                              g * m_group:(g + 1) * m_group],
                                    in_=ps)
                            else:
                                # psum for this group is d2 - ref (ref folded
                                # into the y2 augmentation rows on host), so
                                # exp((ref-d2)/T) needs only the free scale.
                                nc.scalar.activation(
                                    out=epair[:, s2, :], in_=ps,
                                    func=EXP,
                                    scale=-1.0 / SOFT_T,
                                    bias=0.0,
                                    **({"accum_out": rsoft[:, nt:nt + 1]}
                                       if SOFT_ACCUM else {}))
                        # exact rows: fold tree 6144 -> 768 into a KR-stack,
                        # one batched 2x MIN reduce per KR tiles (the fold9
                        # recipe; wide per-tile reduces demote in situ)
                        sme = spair[:, s2, :]
                        u = wpool.tile([P, we // 2], bf16, tag="u",
                                       name="u", bufs=3)
                        nc.vector.tensor_tensor(
                            out=u, in0=sme[:, :we // 2],
                            in1=sme[:, we // 2:], op=MIN)
                        nc.vector.tensor_tensor(
                            out=u[:, :we // 4], in0=u[:, :we // 4],
                            in1=u[:, we // 4:we // 2], op=MIN)
                        nc.vector.tensor_tensor(
                            out=rst[:, nt % KR, :], in0=u[:, :we // 8],
                            in1=u[:, we // 8:we // 4], op=MIN)
                        if nt % KR == KR - 1:
                            nc.vector.tensor_reduce(
                                out=rowmin[:, nt - KR + 1:nt + 1],
                                in_=rst[:, :, :],
                                axis=mybir.AxisListType.X, op=MIN)
                        if s2 == 1:
                            # exact cols: paired 2x TT-MIN (12288 wide)
                            if nt == 1 and rep == 0:
                                nc.vector.tensor_copy(out=cacc_e, in_=spair)
                            else:
                                nc.vector.tensor_tensor(
                                    out=cacc_e, in0=cacc_e, in1=spair,
                                    op=MIN)
                        # soft cols: DVE 2x MAX chain on [0, wdm), gpsimd ADD
                        # chain on [wdm, wsoft) - both slot-alternating
                        if wdm:
                            if nt < 2 and rep == 0:
                                nc.vector.tensor_copy(
                                    out=cacc_m[:, s2, :],
                                    in_=epair[:, s2, :wdm])
                            else:
                                nc.vector.tensor_tensor(
                                    out=cacc_m[:, s2, :],
                                    in0=cacc_m[:, s2, :],
                                    in1=epair[:, s2, :wdm], op=MAX)
                        if wgp:
                            nc.gpsimd.tensor_tensor(
                                out=gacc[:, s2, :], in0=gacc[:, s2, :],
                                in1=epair[:, s2, wdm:], op=ADD)
                # merge pair slots
                nc.vector.tensor_tensor(
                    out=cacc_e[:, 0, :], in0=cacc_e[:, 0, :],
                    in1=cacc_e[:, 1, :], op=MIN)
                if wdm:
                    nc.vector.tensor_tensor(
                        out=cacc_m[:, 0, :], in0=cacc_m[:, 0, :],
                        in1=cacc_m[:, 1, :], op=MAX)
                if wgp:
                    nc.vector.tensor_tensor(
                        out=gacc[:, 0, :], in0=gacc[:, 0, :],
                        in1=gacc[:, 1, :], op=ADD)
                # exact col tail: PE transposes + 2x MIN reduces
                ntile_e = we // P
                tpb = max(1, min(ntile_e, (m_group * 2) // P))
                for t0 in range(0, ntile_e, tpb):
                    cnt = min(tpb, ntile_e - t0)
                    pt = ppool.tile([P, tpb, P], bf16, tag="ps", name="pt")
                    for i in range(cnt):
                        t = t0 + i
                        nc.tensor.transpose(
                            pt[:, i, :], cacc_e[:, 0, t * P:(t + 1) * P],
                            ident)
                    nc.vector.tensor_reduce(
                        out=colmin[:, t0:t0 + cnt], in_=pt[:, :cnt, :],
                        axis=mybir.AxisListType.X, op=MIN)
                # soft col tail: transposes + MAX reduce (cacc_m slice) and
                # ADD reduce (gacc slice); host applies ref - T*ln to both.
                ndm, ngp = wdm // P, wgp // P
                if wdm:
                    ptm = ppool.tile([P, ndm, P], bf16, tag="ps", name="ptm")
                    for i in range(ndm):
                        nc.tensor.transpose(
                            ptm[:, i, :], cacc_m[:, 0, i * P:(i + 1) * P],
                            ident)
                    nc.vector.tensor_reduce(
                        out=colsum_s[:, :ndm], in_=ptm[:, :, :],
                        axis=mybir.AxisListType.X, op=MAX)
                if wgp:
                    pt2 = ppool.tile([P, ngp, P], bf16, tag="ps", name="pt2")
                    for i in range(ngp):
                        nc.tensor.transpose(
                            pt2[:, i, :], gacc[:, 0, i * P:(i + 1) * P],
                            ident)
                    nc.vector.tensor_reduce(
                        out=colsum_s[:, ndm:], in_=pt2[:, :, :],
                        axis=mybir.AxisListType.X, op=ADD)
                nc.sync.dma_start(out[:, 0:nt_count], rowmin[:, :])
                nc.sync.dma_start(
                    out[:, nt_count:2 * nt_count], rsoft[:, :])
                nc.sync.dma_start(
                    out[:, 2 * nt_count:2 * nt_count + ntile_e],
                    colmin[:, :ntile_e])
                nc.sync.dma_start(
                    out[:, 2 * nt_count + ntile_e:
                        2 * nt_count + ntile_e + wsoft // P],
                    colsum_s[:, :])
                final_colacc = None

            if row_mode == "fold12":
                # fold9's paired column chain + the row side as ONE direct
                # tensor_reduce per tile: bf16 MIN tensor_reduce measured at
                # 2 elem/cycle on HW (same rate as the fold tree) but in a
                # single instruction - saves ~4 instruction overheads and the
                # batched-stack bookkeeping.
                cacc2 = cpool.tile([P, 2, n], bf16)
                for rep in range(repeat):
                    for nt in range(nt_count):
                        lhsT = xTs[:kaug, nt * P:(nt + 1) * P]
                        if nt % 2 == 0:
                            spair = wpool.tile([P, 2, n], bf16, tag="sp",
                                               name="spair", bufs=2)
                        for g in range(ngroups):
                            ps = ppool.tile([P, m_group], fp32,
                                            tag="ps", name="ps")
                            for k in range(mm_per_g):
                                nc.tensor.matmul(
                                    ps[:, k * mm_free:(k + 1) * mm_free],
                                    lhsT,
                                    yTs[:kaug,
                                        g * m_group + k * mm_free:
                                        g * m_group + (k + 1) * mm_free],
                                    start=True, stop=True)
                            nc.scalar.copy(
                                out=spair[:, nt % 2,
                                          g * m_group:(g + 1) * m_group],
                                in_=ps)
                        if nt % 2 == 1:
                            if nt == 1 and rep == 0:
                                nc.vector.tensor_copy(out=cacc2, in_=spair)
                            else:
                                nc.vector.tensor_tensor(
                                    out=cacc2, in0=cacc2, in1=spair, op=MIN)
                        nc.vector.tensor_reduce(
                            out=rowmin[:, nt:nt + 1],
                            in_=spair[:, nt % 2, :],
                            axis=mybir.AxisListType.X, op=MIN)
                nc.vector.tensor_tensor(
                    out=cacc2[:, 0, :], in0=cacc2[:, 0, :],
                    in1=cacc2[:, 1, :], op=MIN)
                final_colacc = cacc2[:, 0, :]

            if row_mode == "fold11":
                # fold9's paired column chain + the row side collapsed into
                # ONE DVE instruction per tile: tensor_scalar with a second
                # (accum) output reduces its own out along the free dim with
                # op1 — op0=min(x, +BIG) makes out a copy and accum_out the
                # exact row min. Runs in 4x_2p (single tensor src, bf16,
                # SBUF→SBUF) = 8192/4 cycles, vs the fold tree's ~3840.
                cacc2 = cpool.tile([P, 2, n], bf16)
                for rep in range(repeat):
                    for nt in range(nt_count):
                        lhsT = xTs[:kaug, nt * P:(nt + 1) * P]
                        if nt % 2 == 0:
                            spair = wpool.tile([P, 2, n], bf16, tag="sp",
                                               name="spair", bufs=2)
                        for g in range(ngroups):
                            ps = ppool.tile([P, m_group], fp32,
                                            tag="ps", name="ps")
                            for k in range(mm_per_g):
                                nc.tensor.matmul(
                                    ps[:, k * mm_free:(k + 1) * mm_free],
                                    lhsT,
                                    yTs[:kaug,
                                        g * m_group + k * mm_free:
                                        g * m_group + (k + 1) * mm_free],
                                    start=True, stop=True)
                            nc.scalar.copy(
                                out=spair[:, nt % 2,
                                          g * m_group:(g + 1) * m_group],
                                in_=ps)
                        if nt % 2 == 1:
                            if nt == 1 and rep == 0:
                                nc.vector.tensor_copy(out=cacc2, in_=spair)
                            else:
                                nc.vector.tensor_tensor(
                                    out=cacc2, in0=cacc2, in1=spair, op=MIN)
                        u = wpool.tile([P, n], bf16, tag="u",
                                       name="u", bufs=2)
                        nc.vector.tensor_scalar(
                            out=u,
                            in0=spair[:, nt % 2, :],
                            scalar1=3.0e38,
                            scalar2=None,
                            op0=MIN,
                            op1=MIN,
                            accum_out=rowmin[:, nt:nt + 1])
                nc.vector.tensor_tensor(
                    out=cacc2[:, 0, :], in0=cacc2[:, 0, :],
                    in1=cacc2[:, 1, :], op=MIN)
                final_colacc = cacc2[:, 0, :]

            if row_mode == "fold9":
                # fold4's per-tile tree + a PAIRED column chain: sfull tiles
                # land in [P, 2, n] pair-stacks and the column accumulator is
                # a [P, 2, n] tile (even tiles fold into slot 0, odd into
                # slot 1) updated by ONE flat 16K-wide 2x TT per pair; the
                # two slots merge once at the end. Strided (3D) TTs are
                # avoided everywhere - they demote to 1x on this hardware.
                KR9 = 8
                fw9 = n // 2
                while fw9 > 1024:
                    fw9 //= 2
                cacc2 = cpool.tile([P, 2, n], bf16)
                rst9 = cpool.tile([P, KR9, fw9 // 2], bf16)
                for rep in range(repeat):
                    for nt in range(nt_count):
                        lhsT = xTs[:kaug, nt * P:(nt + 1) * P]
                        if nt % 2 == 0:
                            spair = wpool.tile([P, 2, n], bf16, tag="sp",
                                               name="spair", bufs=2)
                        for g in range(ngroups):
                            ps = ppool.tile([P, m_group], fp32,
                                            tag="ps", name="ps")
                            for k in range(mm_per_g):
                                nc.tensor.matmul(
                                    ps[:, k * mm_free:(k + 1) * mm_free],
                                    lhsT,
                                    yTs[:kaug,
                                        g * m_group + k * mm_free:
                                        g * m_group + (k + 1) * mm_free],
                                    start=True, stop=True)
                            nc.scalar.copy(
                                out=spair[:, nt % 2,
                                          g * m_group:(g + 1) * m_group],
                                in_=ps)
                        if nt % 2 == 1:
                            if nt == 1 and rep == 0:
                                nc.vector.tensor_copy(out=cacc2, in_=spair)
                            else:
                                nc.vector.tensor_tensor(
                                    out=cacc2, in0=cacc2, in1=spair, op=MIN)
                        # per-tile row fold tree (2D contiguous slices only)
                        sme = spair[:, nt % 2, :]
                        u = wpool.tile([P, n // 2], bf16, tag="u",
                                       name="u", bufs=3)
                        nc.vector.tensor_tensor(
                            out=u, in0=sme[:, :n // 2],
                            in1=sme[:, n // 2:], op=MIN)
                        w = n // 2
                        while w > 1024:
                            nc.vector.tensor_tensor(
                                out=u[:, :w // 2], in0=u[:, :w // 2],
                                in1=u[:, w // 2:w], op=MIN)
                            w //= 2
                        nc.vector.tensor_tensor(
                            out=rst9[:, nt % KR9, :], in0=u[:, :w // 2],
                            in1=u[:, w // 2:w], op=MIN)
                        if nt % KR9 == KR9 - 1:
                            nc.vector.tensor_reduce(
                                out=rowmin[:, nt - KR9 + 1:nt + 1],
                                in_=rst9[:, :, :],
                                axis=mybir.AxisListType.X, op=MIN)
                # merge the two interleaved column accumulators in place and
                # expose the result as this mode's colacc for the tail
                nc.vector.tensor_tensor(
                    out=cacc2[:, 0, :], in0=cacc2[:, 0, :],
                    in1=cacc2[:, 1, :], op=MIN)
                final_colacc = cacc2[:, 0, :]

            if row_mode == "fold7":
                # fold4's tree shape with all sub-L1 levels BATCHED across 4
                # tiles: per tile only [colacc TT, L1-into-stack]; per 4 tiles
                # one batched TT per tree level (3D APs over [P, 4, w]
                # stacks); per 8 tiles one batched 1x reduce. Minimizes DVE
                # instruction count (~3.4/tile) at unchanged 2x busy work.
                KB7 = 4
                KR7 = 8
                ust = cpool.tile([P, KB7, n // 2], bf16)
                vst7 = cpool.tile([P, KB7, n // 4], bf16)
                wst7 = cpool.tile([P, KB7, n // 8], bf16)
                qst7 = cpool.tile([P, KB7, n // 16], bf16)
                rst7 = cpool.tile([P, KR7, n // 32], bf16)
                for rep in range(repeat):
                    for nt in range(nt_count):
                        lhsT = xTs[:kaug, nt * P:(nt + 1) * P]
                        sfull = wpool.tile([P, n], bf16, tag="s",
                                           name="sfull", bufs=3)
                        for g in range(ngroups):
                            ps = ppool.tile([P, m_group], fp32,
                                            tag="ps", name="ps")
                            for k in range(mm_per_g):
                                nc.tensor.matmul(
                                    ps[:, k * mm_free:(k + 1) * mm_free],
                                    lhsT,
                                    yTs[:kaug,
                                        g * m_group + k * mm_free:
                                        g * m_group + (k + 1) * mm_free],
                                    start=True, stop=True)
                            nc.scalar.copy(
                                out=sfull[:, g * m_group:(g + 1) * m_group],
                                in_=ps)
                        if nt == 0 and rep == 0:
                            nc.vector.tensor_copy(out=colacc, in_=sfull)
                        else:
                            nc.vector.tensor_tensor(
                                out=colacc, in0=colacc, in1=sfull, op=MIN)
                        nc.vector.tensor_tensor(
                            out=ust[:, nt % KB7, :], in0=sfull[:, :n // 2],
                            in1=sfull[:, n // 2:], op=MIN)
                        if nt % KB7 == KB7 - 1:
                            nc.vector.tensor_tensor(
                                out=vst7, in0=ust[:, :, :n // 4],
                                in1=ust[:, :, n // 4:], op=MIN)
                            nc.vector.tensor_tensor(
                                out=wst7, in0=vst7[:, :, :n // 8],
                                in1=vst7[:, :, n // 8:], op=MIN)
                            nc.vector.tensor_tensor(
                                out=qst7, in0=wst7[:, :, :n // 16],
                                in1=wst7[:, :, n // 16:], op=MIN)
                            r0 = (nt - KB7 + 1) % KR7
                            nc.vector.tensor_tensor(
                                out=rst7[:, r0:r0 + KB7, :],
                                in0=qst7[:, :, :n // 32],
                                in1=qst7[:, :, n // 32:], op=MIN)
                        if nt % KR7 == KR7 - 1:
                            nc.vector.tensor_reduce(
                                out=rowmin[:, nt - KR7 + 1:nt + 1],
                                in_=rst7[:, :, :],
                                axis=mybir.AxisListType.X, op=MIN)

            if row_mode == "fold5":
                # Fewest DVE ops per tile: colacc TT, tree L1 (8192->4096),
                # tree L2 into a per-tile stack slot (4096->2048), and ONE
                # batched 1x tensor_reduce per 4 tiles over the [P, 4, 2048]
                # stack producing 4 row minima at once.
                KR5 = 4
                for rep in range(repeat):
                    for nt in range(nt_count):
                        lhsT = xTs[:kaug, nt * P:(nt + 1) * P]
                        sfull = wpool.tile([P, n], bf16, tag="s",
                                           name="sfull", bufs=3)
                        for g in range(ngroups):
                            ps = ppool.tile([P, m_group], fp32,
                                            tag="ps", name="ps")
                            for k in range(mm_per_g):
                                nc.tensor.matmul(
                                    ps[:, k * mm_free:(k + 1) * mm_free],
                                    lhsT,
                                    yTs[:kaug,
                                        g * m_group + k * mm_free:
                                        g * m_group + (k + 1) * mm_free],
                                    start=True, stop=True)
                            dstg = sfull[:, g * m_group:(g + 1) * m_group]
                            if g >= ngroups - dve_copies:
                                nc.vector.tensor_copy(out=dstg, in_=ps)
                            else:
                                nc.scalar.copy(out=dstg, in_=ps)
                        if nt == 0 and rep == 0:
                            nc.vector.tensor_copy(out=colacc, in_=sfull)
                        else:
                            nc.vector.tensor_tensor(
                                out=colacc, in0=colacc, in1=sfull, op=MIN)
                        u = wpool.tile([P, n // 2], bf16, tag="u",
                                       name="u", bufs=3)
                        nc.vector.tensor_tensor(
                            out=u, in0=sfull[:, :n // 2],
                            in1=sfull[:, n // 2:], op=MIN)
                        if nt % KR5 == 0:
                            vst = wpool.tile([P, KR5, n // 4], bf16,
                                             tag="v5", name="vst", bufs=2)
                        nc.vector.tensor_tensor(
                            out=vst[:, nt % KR5, :], in0=u[:, :n // 4],
                            in1=u[:, n // 4:], op=MIN)
                        if nt % KR5 == KR5 - 1:
                            nc.vector.tensor_reduce(
                                out=rowmin[:, nt - KR5 + 1:nt + 1],
                                in_=vst[:, :, :],
                                axis=mybir.AxisListType.X, op=MIN)

            row_mode4 = row_mode == "fold4"
            if row_mode in ("fold", "fold4"):
                # One n-wide s tile per n-tile: ONE wide col-min TT, and row
                # mins via a fold tree of wide TT-mins and one small reduce.
                # "fold4": fold one level deeper (to 256) and batch the 1x
                # tensor_reduce across KR tiles via a [P, KR, 256] stack.
                fold_to = 256 if row_mode4 else 512
                KR = 8
                if row_mode4:
                    rstack = cpool.tile([P, KR, fold_to], bf16)
                for rep in range(repeat):
                    for nt in range(nt_count):
                        lhsT = xTs[:kaug, nt * P:(nt + 1) * P]
                        sfull = wpool.tile([P, n], bf16, tag="s",
                                           name="sfull", bufs=3)
                        for g in range(ngroups):
                            ps = ppool.tile([P, m_group], fp32,
                                            tag="ps", name="ps")
                            for k in range(mm_per_g):
                                nc.tensor.matmul(
                                    ps[:, k * mm_free:(k + 1) * mm_free],
                                    lhsT,
                                    yTs[:kaug,
                                        g * m_group + k * mm_free:
                                        g * m_group + (k + 1) * mm_free],
                                    start=True, stop=True)
                            dstg = sfull[:, g * m_group:(g + 1) * m_group]
                            if g >= ngroups - dve_copies:
                                nc.vector.tensor_copy(out=dstg, in_=ps)
                            else:
                                nc.scalar.copy(out=dstg, in_=ps)
                        if nt == 0 and rep == 0:
                            nc.vector.tensor_copy(out=colacc, in_=sfull)
                        else:
                            nc.vector.tensor_tensor(
                                out=colacc, in0=colacc, in1=sfull, op=MIN)
                        # row fold tree
                        u = wpool.tile([P, n // 2], bf16, tag="u",
                                       name="u", bufs=3)
                        nc.vector.tensor_tensor(
                            out=u, in0=sfull[:, :n // 2],
                            in1=sfull[:, n // 2:], op=MIN)
                        w = n // 2
                        while w > 2 * fold_to:
                            nc.vector.tensor_tensor(
                                out=u[:, :w // 2], in0=u[:, :w // 2],
                                in1=u[:, w // 2:w], op=MIN)
                            w //= 2
                        if row_mode4:
                            # last fold lands in this tile's reduce-stack slot
                            nc.vector.tensor_tensor(
                                out=rstack[:, nt % KR, :], in0=u[:, :w // 2],
                                in1=u[:, w // 2:w], op=MIN)
                            if nt % KR == KR - 1:
                                nc.vector.tensor_reduce(
                                    out=rowmin[:, nt - KR + 1:nt + 1],
                                    in_=rstack[:, :, :],
                                    axis=mybir.AxisListType.X, op=MIN)
                        else:
                            nc.vector.tensor_reduce(
                                out=rowmin[:, nt:nt + 1], in_=u[:, :w],
                                axis=mybir.AxisListType.X, op=MIN)

            for rep in range(
                    repeat
                    if row_mode not in ("fold", "fold4", "fold5", "fold7",
                                        "fold9", "fold10", "fold11", "fold12", "fold17", "fold2",
                                        "ttrh", "max8")
                    else 0):
              for nt in range(nt_count):
                lhsT = xTs[:kaug, nt * P:(nt + 1) * P]
                for g in range(ngroups):
                    ps = ppool.tile([P, m_group], fp32, tag="ps", name="ps")
                    for k in range(mm_per_g):
                        nc.tensor.matmul(
                            ps[:, k * mm_free:(k + 1) * mm_free],
                            lhsT,
                            yTs[:kaug, g * m_group + k * mm_free:
                                g * m_group + (k + 1) * mm_free],
                            start=True,
                            stop=True,
                        )
                    s = wpool.tile([P, m_group], bf16, name="s")
                    nc.scalar.copy(out=s, in_=ps)

                    # column-min accumulator (n folded into the 128 lanes)
                    csl = colacc[:, g * m_group:(g + 1) * m_group]
                    if nt == 0:
                        nc.vector.tensor_copy(out=csl, in_=s)
                    else:
                        nc.vector.tensor_tensor(out=csl, in0=csl, in1=s, op=MIN)

                    # row mins
                    if row_mode == "ttr2":
                        # like "ttr" but ping-pongs the elementwise-min
                        # accumulator to avoid in-place out/in1 aliasing
                        accs = [rowacc, rowacc2]
                        dst = accs[g % 2]
                        src = s if g == 0 else accs[1 - g % 2]
                        nc.vector.tensor_tensor_reduce(
                            out=dst,
                            in0=s,
                            in1=src,
                            scale=1.0,
                            scalar=3.0e38,
                            op0=MIN,
                            op1=MIN,
                            accum_out=rowmin[:, nt:nt + 1],
                        )
                    elif row_mode == "ttr":
                        # rowacc = min(rowacc, s) elementwise; accum_out gets
                        # min over the free dim of the updated rowacc. The
                        # last group's accum covers all m -> true row min.
                        nc.vector.tensor_tensor_reduce(
                            out=rowacc,
                            in0=s,
                            in1=(s if g == 0 else rowacc),
                            scale=1.0,
                            scalar=3.0e38,
                            op0=MIN,
                            op1=MIN,
                            accum_out=rowmin[:, nt:nt + 1],
                        )
                    else:
                        for k in range(mm_per_g):
                            ssl = s[:, k * mm_free:(k + 1) * mm_free]
                            if g == 0 and k == 0:
                                nc.vector.tensor_copy(out=rowacc_narrow, in_=ssl)
                            else:
                                nc.vector.tensor_tensor(
                                    out=rowacc_narrow, in0=rowacc_narrow,
                                    in1=ssl, op=MIN)
                        if g == ngroups - 1:
                            nc.vector.tensor_reduce(
                                out=rowmin[:, nt:nt + 1], in_=rowacc_narrow,
                                axis=mybir.AxisListType.X, op=MIN)

            # column-extreme finish: transpose each [128, 128] block of
            # colacc on PE, then reduce the (former partition) lanes on DVE.
            # (fold17 runs its own tail + DMA inside its mode block.)
            TAILOP = mybir.AluOpType.max if row_mode == "max8" else MIN
            if row_mode == "fold17":
                pass
            elif col_tail == "host":
                nc.sync.dma_start(colout[:, :], final_colacc[:, :])
            elif not skip_tail:
                # batch transposes into wide bf16 PSUM tiles so the lane-min
                # runs as a few wide DVE reduces instead of nt_count small ones
                tpb = max(1, min(nt_count, (m_group * 2) // P))
                for t0 in range(0, nt_count, tpb):
                    cnt = min(tpb, nt_count - t0)
                    pt = ppool.tile([P, tpb, P], bf16, tag="ps", name="pt")
                    for i in range(cnt):
                        t = t0 + i
                        nc.tensor.transpose(
                            pt[:, i, :], final_colacc[:, t * P:(t + 1) * P], ident)
                    nc.vector.tensor_reduce(
                        out=colmin[:, t0:t0 + cnt], in_=pt[:, :cnt, :],
                        axis=mybir.AxisListType.X, op=TAILOP)
            else:
                nc.vector.tensor_copy(out=colmin, in_=rowmin)

            if row_mode == "fold17":
                pass
            elif row_mode == "max8":
                nc.sync.dma_start(out[:, :rows_w], rowtop[:, :, :])
            else:
                nc.sync.dma_start(out[:, :rows_w], rowmin[:, :])
            if row_mode != "fold17" and col_tail != "host":
                nc.sync.dma_start(out[:, rows_w:], colmin[:, :])

    nc.finalize()  # runs the Bacc compile passes (event sems, reg alloc, ...)
    return nc


def _prep_inputs(x, y, kaug=KAUG, negate=False, soft_ref=None):
    """Build the augmented, transposed bf16 operands for each batch.

    negate=True flips every matmul contribution so psum = -d2 (for the
    Max-instruction row mode, where minima become maxima).
    For fold17, the softmin reference is folded into the y2 rows of the
    LAST m-group's columns so the device psum there is d2 - ref."""
    bf = ml_dtypes.bfloat16
    sg = -1.0 if negate else 1.0
    if soft_ref is None:
        mode = os.environ.get("CHAMFER_ROW_MODE", DEFAULT_ROW_MODE)
        soft_ref = SOFT_REF if mode == "fold17" else 0.0
    in_maps = []
    for b in range(x.shape[0]):
        xb = np.asarray(x[b], dtype=np.float32)
        yb = np.asarray(y[b], dtype=np.float32)
        n = xb.shape[0]
        x2 = np.sum(xb * xb, axis=-1)
        y2 = np.sum(yb * yb, axis=-1)
        if soft_ref:
            y2 = y2.copy()
            y2[-2048:] -= soft_ref
        x2_hi = (sg * x2).astype(bf)
        x2_lo = (sg * x2 - x2_hi.astype(np.float32)).astype(bf)
        y2_hi = (sg * y2).astype(bf)
        y2_lo = (sg * y2 - y2_hi.astype(np.float32)).astype(bf)
        ones = np.ones((1, n), dtype=bf)
        xT = np.concatenate(
            [xb.T.astype(bf), ones, ones, x2_hi[None], x2_lo[None]], axis=0)
        yT = np.concatenate(
            [(sg * -2.0 * yb).T.astype(bf), y2_hi[None], y2_lo[None], ones,
             ones],
            axis=0)
        if kaug > KAUG:
            pad = np.zeros((kaug - KAUG, n), dtype=bf)
            xT = np.concatenate([xT, pad], axis=0)
            yT = np.concatenate([yT, pad], axis=0)
        in_maps.append({
            "xT": np.ascontiguousarray(xT),
            "yT": np.ascontiguousarray(yT),
        })
    return in_maps


def _postprocess(results, n=N, row_mode="fold"):
    nt_count = n // P
    rows_w = nt_count * 8 if row_mode == "max8" else nt_count
    total = 0.0
    nb = len(results)
    if row_mode == "fold17":
        # out layout: [rowmin_e 64 | rsoft 64 | colmin_e 48 | colsum_s 16]
        we = n - 2048
        ne, ns = we // P, 2048 // P
        for b in range(nb):
            o = np.asarray(results[b]["out"], dtype=np.float64)
            rowmin_e = o[:, :nt_count].T.reshape(-1)          # exact, d2
            rsoft = o[:, nt_count:2 * nt_count].T.reshape(-1)  # sum exp
            with np.errstate(divide="ignore"):
                d_soft = SOFT_REF - SOFT_T * np.log(
                    np.maximum(rsoft, 1e-300))
            rowmin = np.minimum(rowmin_e, d_soft)
            colmin_e = o[:, 2 * nt_count:2 * nt_count + ne].T.reshape(-1)
            csum = o[:, 2 * nt_count + ne:
                     2 * nt_count + ne + ns].T.reshape(-1)
            with np.errstate(divide="ignore"):
                col_soft = SOFT_REF - SOFT_T * np.log(
                    np.maximum(csum, 1e-300))
            total += np.sqrt(np.maximum(rowmin, 0.0)).sum()
            total += np.sqrt(np.maximum(colmin_e, 0.0)).sum()
            total += np.sqrt(np.maximum(col_soft, 0.0)).sum()
        loss = total / nb / n
        return np.asarray(loss, dtype=np.float32)
    for b in range(nb):
        o = np.asarray(results[b]["out"], dtype=np.float64)
        if row_mode == "max8":
            # out[p, t*8+j] = j-th largest of -d2 for row n=t*128+p
            rowmin = -o[:, :rows_w].reshape(P, nt_count, 8)[:, :, 0]
            rowmin = rowmin.T.reshape(-1)
        else:
            rowmin = o[:, :rows_w].T.reshape(-1)   # [n], index t*128+p
        if "colout" in results[b]:
            co = np.asarray(results[b]["colout"], dtype=np.float32)
            colmin = co.min(axis=0).astype(np.float64)
        else:
            colmin = o[:, rows_w:].T.reshape(-1)
            if row_mode == "max8":
                colmin = -colmin
        total += np.sqrt(np.maximum(rowmin, 0.0)).sum()
        total += np.sqrt(np.maximum(colmin, 0.0)).sum()
    loss = total / nb / n
    return np.asarray(loss, dtype=np.float32)


def _get_runner(n_cores=B, row_mode=None):
    """Build the Bass module once and return a reusable jitted runner.

    Modeled on concourse.bass2jax.run_bass_via_pjrt's multi-core branch, but
    keeps the jitted callable so repeated invocations don't re-lower."""
    if row_mode is None:
        row_mode = os.environ.get("CHAMFER_ROW_MODE", DEFAULT_ROW_MODE)
    key = ("runner", n_cores, row_mode)
    if key in _CACHE:
        return _CACHE[key]

    import jax
    from jax.experimental.shard_map import shard_map
    from jax.sharding import Mesh, PartitionSpec
    from concourse import bass2jax, mybir

    nc = _build_nc(row_mode=row_mode,
                   col_tail=os.environ.get("CHAMFER_COL_TAIL", "device"),
                   dve_copies=int(os.environ.get("CHAMFER_DVE_COPIES", "0")))

    bass2jax.install_neuronx_cc_hook()
    assert nc.dbg_addr is None

    partition_name = (
        nc.partition_id_tensor.name if nc.partition_id_tensor else None)
    in_names, out_names, out_avals = [], [], []
    for alloc in nc.m.functions[0].allocations:
        if not isinstance(alloc, mybir.MemoryLocationSet):
            continue
        name = alloc.memorylocations[0].name
        if alloc.kind == "ExternalInput":
            if name != partition_name:
                in_names.append(name)
        elif alloc.kind == "ExternalOutput":
            out_names.append(name)
            out_avals.append(jax.core.ShapedArray(
                tuple(alloc.tensor_shape), mybir.dt.np(alloc.dtype)))
    n_params = len(in_names)
    n_outs = len(out_avals)
    all_in_names = list(in_names) + list(out_names)
    if partition_name is not None:
        all_in_names.append(partition_name)
    donate = tuple(range(n_params, n_params + n_outs))

    def _body(*args):
        operands = list(args)
        if partition_name is not None:
            operands.append(bass2jax.partition_id_tensor())
        outs = bass2jax._bass_exec_p.bind(
            *operands,
            out_avals=tuple(out_avals),
            in_names=tuple(all_in_names),
            out_names=tuple(out_names),
            lowering_input_output_aliases=(),
            sim_require_finite=True,
            sim_require_nnan=True,
            nc=nc,
        )
        return tuple(outs)

    devices = jax.devices()[:n_cores]
    mesh = Mesh(np.asarray(devices), ("core",))
    sharded = jax.jit(
        shard_map(
            _body, mesh=mesh,
            in_specs=(PartitionSpec("core"),) * (n_params + n_outs),
            out_specs=(PartitionSpec("core"),) * n_outs,
            check_rep=False,
        ),
        donate_argnums=donate,
        keep_unused=True,
    )

    def run(in_maps):
        per_core = [[np.asarray(m[nm]) for nm in in_names] for m in in_maps]
        concat_in = [
            np.concatenate([per_core[c][i] for c in range(n_cores)], axis=0)
            for i in range(n_params)
        ]
        concat_zeros = [
            np.zeros((n_cores * a.shape[0], *a.shape[1:]), a.dtype)
            for a in out_avals
        ]
        out_arrs = sharded(*concat_in, *concat_zeros)
        jax.block_until_ready(out_arrs)
        return [
            {nm: np.asarray(out_arrs[i]).reshape(
                n_cores, *out_avals[i].shape)[c]
             for i, nm in enumerate(out_names)}
            for c in range(n_cores)
        ]

    _CACHE[key] = run
    return run


def kernel(x, y):
    import time

    row_mode = os.environ.get("CHAMFER_ROW_MODE", DEFAULT_ROW_MODE)
    x = np.asarray(x)
    y = np.asarray(y)
    in_maps = _prep_inputs(x, y, negate=(row_mode == "max8"))
    run = _get_runner(n_cores=len(in_maps), row_mode=row_mode)
    # the device occasionally wedges transiently on a fresh NEFF's first
    # execution (NRT_EXEC_UNIT_UNRECOVERABLE); a retry reliably clears it
    last_err = None
    for attempt in range(4):
        try:
            results = run(in_maps)
            return _postprocess(results, row_mode=row_mode)
        except Exception as e:  # noqa: BLE001 - retry any runtime failure
            last_err = e
            time.sleep(2.0)
            try:
                import jax
                jax.clear_caches()
            except Exception:
                pass
            _CACHE.clear()  # rebuild runner; NEFF recompile is disk-cached
            run = _get_runner(n_cores=len(in_maps), row_mode=row_mode)
    raise last_err



# revision 28
# speedup vs baseline: 1.9316x; 1.9316x over previous
"""Chamfer loss kernel for Trainium2 (8 NeuronCores, one batch per core).

Problem: B=8, N=M=8192, D=64 fp32.
  rd = pairwise euclidean distances x[b] vs y[b]   [B, N, M]
  loss = mean_b( sum_n min_m rd + sum_m min_n rd ) / M

Device strategy (per core = one batch):
  - sqrt is monotonic -> only need minima of SQUARED distances; sqrt+sums
    happen on host over 2*8192 values per batch.
  - d2 = x2 + y2 - 2*x.y is produced entirely by ONE bf16 matmul with an
    augmented contraction dim:
       lhsT rows (x side, [68, N]): [x_d (64) ; 1 ; 1 ; x2_hi ; x2_lo]
       rhs  rows (y side, [68, M]): [-2*y_d (64) ; y2_hi ; y2_lo ; 1 ; 1]
    so psum = sum_d x_d*(-2 y_d) + y2_hi + y2_lo + x2_hi + x2_lo = d2.
    (hi/lo bf16 splits keep the squared-norm terms at ~fp24 precision.)
  - ScalarE copies each PSUM group into bf16 SBUF tiles; VectorE (the
    bottleneck engine, bf16 tensor_tensor min at 2 elem/cycle/lane) does
    all min work. The default "fold9" schedule minimizes DVE instruction
    COUNT at constant 2x busy work (each DVE instruction was measured to
    carry ~0.5-1us of hidden cost beyond its streaming time):
      * sfull tiles land in [P, 2, n] PAIR stacks; the column accumulator
        is a [P, 2, n] tile (even tiles fold into slot 0, odd into slot 1)
        updated by ONE flat 16K-wide 2x TT per pair, merged once at the end.
      * per-tile row fold tree of 2D contiguous TT-mins down to 512, the
        last fold writing a [P, 8, 512] stack; ONE batched 1x tensor_reduce
        per 8 tiles. (Strided/3D TTs demote to 1x on HW - never use them;
        small per-tile reduces cost ~1us hidden each - batch them.)
  - The col accumulator is finished by PE transposes + wide DVE reduces.
Host does the final sqrt / sums / mean in float64.

The default "fold17" schedule additionally offloads a quarter of the DVE
min work to the Scalar (ACT) engine via an exp-domain trick: the LAST
2048-wide m-group of each tile is evacuated from PSUM not by a copy but
by ONE ScalarE activation Exp (measured ~2 elem/cycle from PSUM, about
as cheap as the copy) computing e = exp((ref-d2)/T) (ref is folded into
the y2 augmentation rows, T into the ACT free scale), with accum_out
producing the group's softmin row partial sums as a free second output:
  * that group's ROW work costs the DVE nothing (accum_out on ACT);
    host recovers d2_soft = ref - T*ln(sum) and takes
    min(exact_rowmin, d2_soft). T=2 keeps the softmin bias ~1e-3.
  * that group's COLUMN chain runs as a 2x TT-MAX in exp domain (exp is
    monotone decreasing -> col MAX = col min of d2, near-exact: only
    bf16 quantization of e, ~4e-3 in d2 units).
  * the exact 6144-wide part keeps fold9's recipe (3-level fold tree to
    768 + one batched 2x MIN tensor_reduce per 8 tiles; paired 2x
    TT-MIN column chain).
Measured (repeat-module min-stats): fold9 549us -> fold17 399us,
rel_err 3.9e-04. HW quirks found by microbenchmark (chained, per-8192
bf16 elems): TT min/max 2x (3.9us) but TT ADD only 1x (14us); MIN
tensor_reduce 2x (4.4us) but MAX/fp16/ADD reduce 1x; tensor_scalar with
accum_out ~25us (fold11 regressed); a WIDE per-tile tensor_reduce
demotes in situ (fold12 regressed) - only the batched stack reduce is
fast; gpsimd TT-ADD ~5ns/elem and stalls the pipeline when put on the
column chain (WGP>0 regressed 546->653us). Rejected earlier:
tensor_tensor_reduce (crashes runtime), vector.max top-8, gpsimd/DMA-CCE
min (no ucode), single-buffer paired PSUM copies (PE p-state throttle).
"""

import os

import numpy as np
import ml_dtypes

P = 128
N = 8192
D = 64
KAUG = D + 4  # 68
B = 8

_CACHE = {}

DEFAULT_ROW_MODE = "fold17"


SOFT_T = float(os.environ.get("CHAMFER_SOFT_T", "2.0"))
SOFT_REF = float(os.environ.get("CHAMFER_SOFT_REF", "10.0"))
SOFT_W = int(os.environ.get("CHAMFER_SOFT_W", "4096"))   # soft m-width
WGP = int(os.environ.get("CHAMFER_WGP", "0"))            # gpsimd col width
SOFT_ACCUM = int(os.environ.get("CHAMFER_SOFT_ACCUM", "1"))


def _build_nc(n=N, mm_free=512, m_group=2048, row_mode="ttr", kaug=KAUG,
              skip_tail=False, repeat=1, col_tail="device", dve_copies=0):
    import concourse.bass as bass
    import concourse.mybir as mybir
    import concourse.tile as tile
    from concourse import bacc
    from concourse.masks import make_identity

    fp32 = mybir.dt.float32
    bf16 = mybir.dt.bfloat16
    MIN = mybir.AluOpType.min

    nt_count = n // P          # n-tiles (output partition blocks)
    ngroups = n // m_group     # m groups per n-tile
    mm_per_g = m_group // mm_free

    # Bacc (not raw Bass): its compile pipeline lowers instructions with more
    # sync waits than the ISA's embedded slots into EventSemaphore insts.
    nc = bacc.Bacc("TRN2", target_bir_lowering=False, debug=False)
    xT = nc.dram_tensor("xT", [kaug, n], bf16, kind="ExternalInput")
    yT = nc.dram_tensor("yT", [kaug, n], bf16, kind="ExternalInput")
    rows_w = nt_count * 8 if row_mode == "max8" else nt_count
    if row_mode == "fold17":
        # [rowmin_e 64 | rsoft 64 | colmin_e 48 | colsum_soft 16]
        out = nc.dram_tensor("out", [P, 192], fp32, kind="ExternalOutput")
    else:
        out = nc.dram_tensor("out", [P, rows_w + nt_count], fp32,
                             kind="ExternalOutput")
    colout = None
    if col_tail == "host":
        # ship the lane-folded col accumulator; host does the 128-lane min
        colout = nc.dram_tensor("colout", [P, n], bf16, kind="ExternalOutput")

    with tile.TileContext(nc) as tc:
        with (
            tc.tile_pool(name="const", bufs=1) as cpool,
            tc.tile_pool(name="work", bufs=3) as wpool,
            tc.tile_pool(name="psum", bufs=2, space="PSUM") as ppool,
        ):
            xTs = cpool.tile([P, n], bf16)
            yTs = cpool.tile([P, n], bf16)
            colacc = (cpool.tile([P, n], bf16, name="colacc")
                      if row_mode not in ("fold9", "fold10", "fold11", "fold12", "fold17")
                      else None)
            rowacc = (cpool.tile([P, m_group], bf16, name="rowacc")
                      if row_mode in ("ttr", "ttr2", "tt") else None)
            rowmin = cpool.tile([P, nt_count], fp32)
            if col_tail != "host":
                colmin = cpool.tile([P, nt_count], fp32)
                ident = cpool.tile([P, P], bf16)

            # chunked loads so early matmuls start before the full tensors land
            n_chunks = max(1, n // 2048)
            cw = n // n_chunks
            for c in range(n_chunks):
                nc.sync.dma_start(
                    xTs[:kaug, c * cw:(c + 1) * cw], xT[:, c * cw:(c + 1) * cw])
                nc.sync.dma_start(
                    yTs[:kaug, c * cw:(c + 1) * cw], yT[:, c * cw:(c + 1) * cw])
            if col_tail != "host":
                make_identity(nc, ident)

            if row_mode == "tt":
                rowacc_narrow = cpool.tile([P, mm_free], bf16)
            if row_mode == "ttr2":
                rowacc2 = cpool.tile([P, m_group], bf16)

            if row_mode == "fold2":
                # alias-free variant of "fold": ping-pong col accumulators and
                # alternate row-tree scratch tiles, in case in-place operands
                # demote the DVE from 2x_1P to 1x mode.
                colacc2 = cpool.tile([P, n], bf16)
                accs = [colacc, colacc2]
                vtile = cpool.tile([P, n // 4], bf16)
                for rep in range(repeat):
                    for nt in range(nt_count):
                        lhsT = xTs[:kaug, nt * P:(nt + 1) * P]
                        sfull = wpool.tile([P, n], bf16, tag="s",
                                           name="sfull", bufs=3)
                        for g in range(ngroups):
                            ps = ppool.tile([P, m_group], fp32,
                                            tag="ps", name="ps")
                            for k in range(mm_per_g):
                                nc.tensor.matmul(
                                    ps[:, k * mm_free:(k + 1) * mm_free],
                                    lhsT,
                                    yTs[:kaug,
                                        g * m_group + k * mm_free:
                                        g * m_group + (k + 1) * mm_free],
                                    start=True, stop=True)
                            nc.scalar.copy(
                                out=sfull[:, g * m_group:(g + 1) * m_group],
                                in_=ps)
                        i = (rep * nt_count + nt) % 2
                        if nt == 0 and rep == 0:
                            nc.vector.tensor_copy(out=accs[i], in_=sfull)
                        else:
                            nc.vector.tensor_tensor(
                                out=accs[i], in0=accs[1 - i], in1=sfull,
                                op=MIN)
                        # row fold tree, alternating scratch tiles (no alias)
                        u = wpool.tile([P, n // 2], bf16, tag="u",
                                       name="u", bufs=3)
                        nc.vector.tensor_tensor(
                            out=u, in0=sfull[:, :n // 2],
                            in1=sfull[:, n // 2:], op=MIN)
                        nc.vector.tensor_tensor(
                            out=vtile, in0=u[:, :n // 4],
                            in1=u[:, n // 4:], op=MIN)
                        nc.vector.tensor_tensor(
                            out=u[:, :n // 8], in0=vtile[:, :n // 8],
                            in1=vtile[:, n // 8:], op=MIN)
                        nc.vector.tensor_tensor(
                            out=vtile[:, :n // 16], in0=u[:, :n // 16],
                            in1=u[:, n // 16:n // 8], op=MIN)
                        nc.vector.tensor_reduce(
                            out=rowmin[:, nt:nt + 1], in_=vtile[:, :n // 16],
                            axis=mybir.AxisListType.X, op=MIN)
                final_colacc = accs[(repeat * nt_count - 1) % 2]
            elif row_mode in ("fold9", "fold10", "fold11", "fold12", "fold17"):
                final_colacc = None  # set after the mode's loop builds it
            else:
                final_colacc = colacc

            if row_mode == "ttrh":
                # One n-wide s tile per n-tile: ONE wide col-min TT, and the
                # ENTIRE row reduction as ONE tensor_tensor_reduce: the
                # elementwise out is the min of the two m-halves (first fold
                # level) and accum_out is the running min over the free dim of
                # that out — i.e. the true row min. Saves the fold tree and
                # the 1x tensor_reduce of the "fold" mode.
                for rep in range(repeat):
                    for nt in range(nt_count):
                        lhsT = xTs[:kaug, nt * P:(nt + 1) * P]
                        sfull = wpool.tile([P, n], bf16, tag="s",
                                           name="sfull", bufs=3)
                        for g in range(ngroups):
                            ps = ppool.tile([P, m_group], fp32,
                                            tag="ps", name="ps")
                            for k in range(mm_per_g):
                                nc.tensor.matmul(
                                    ps[:, k * mm_free:(k + 1) * mm_free],
                                    lhsT,
                                    yTs[:kaug,
                                        g * m_group + k * mm_free:
                                        g * m_group + (k + 1) * mm_free],
                                    start=True, stop=True)
                            nc.scalar.copy(
                                out=sfull[:, g * m_group:(g + 1) * m_group],
                                in_=ps)
                        if nt == 0 and rep == 0:
                            nc.vector.tensor_copy(out=colacc, in_=sfull)
                        else:
                            nc.vector.tensor_tensor(
                                out=colacc, in0=colacc, in1=sfull, op=MIN)
                        u = wpool.tile([P, n // 2], bf16, tag="u",
                                       name="u", bufs=3)
                        nc.vector.tensor_tensor_reduce(
                            out=u,
                            in0=sfull[:, :n // 2],
                            in1=sfull[:, n // 2:],
                            scale=1.0,
                            scalar=3.0e38,
                            op0=MIN,
                            op1=MIN,
                            accum_out=rowmin[:, nt:nt + 1],
                        )

            if row_mode == "max8":
                # Inputs are NEGATED (-d2, see _prep_inputs(negate=True)) so
                # minima become maxima and the DVE's Max instruction (top-8
                # per partition in one op, free size up to 16K) computes each
                # n-tile's row extreme in ONE instruction, replacing the fold
                # tree + 1x tensor_reduce. Col side is a wide TT-max chain.
                # PSUM->SBUF copies are PAIRED (one 4096-wide ScalarE copy per
                # two m-groups) out of a single full-PSUM [P, 2, m_group]
                # tile: PE and ScalarE serialize on the 8 banks, but their
                # combined 7.4us/tile cadence stays under the DVE's ~9us.
                MAX = mybir.AluOpType.max
                rowtop = cpool.tile([P, nt_count, 8], fp32)
                for rep in range(repeat):
                    for nt in range(nt_count):
                        lhsT = xTs[:kaug, nt * P:(nt + 1) * P]
                        sfull = wpool.tile([P, n], bf16, tag="s",
                                           name="sfull", bufs=3)
                        for g in range(ngroups):
                            ps = ppool.tile([P, m_group], fp32,
                                            tag="ps", name="ps")
                            for k in range(mm_per_g):
                                nc.tensor.matmul(
                                    ps[:, k * mm_free:(k + 1) * mm_free],
                                    lhsT,
                                    yTs[:kaug,
                                        g * m_group + k * mm_free:
                                        g * m_group + (k + 1) * mm_free],
                                    start=True, stop=True)
                            dst = sfull[:, g * m_group:(g + 1) * m_group]
                            if g >= ngroups - dve_copies:
                                nc.vector.tensor_copy(out=dst, in_=ps)
                            else:
                                nc.scalar.copy(out=dst, in_=ps)
                        if nt == 0 and rep == 0:
                            nc.vector.tensor_copy(out=colacc, in_=sfull)
                        else:
                            nc.vector.tensor_tensor(
                                out=colacc, in0=colacc, in1=sfull, op=MAX)
                        nc.vector.max(rowtop[:, nt, :], sfull)

            if row_mode == "fold10":
                # fold9 + interleaved pair layout: the ScalarE copies place
                # the two tiles' m-groups interleaved in a flat [P, 2n] pair
                # buffer (block 2g+t = tile t, group g), so the first
                # log2(ngroups) row-fold levels are ALSO single flat 2x TTs
                # per PAIR (halves of the region pair elements of the same
                # tile). ~3.6 DVE instructions per tile. The column pair
                # accumulator keeps the interleaved layout and is merged into
                # natural m order by ngroups small TTs once at the end.
                KRA = 8
                wpt = m_group
                while wpt > 1024:
                    wpt //= 2
                cacc2 = cpool.tile([P, 2 * n], bf16)
                colaccX = cpool.tile([P, n], bf16)
                rstA = cpool.tile([P, KRA, wpt // 2], bf16)
                npair_folds = 0
                gg = ngroups
                while gg > 1:
                    gg //= 2
                    npair_folds += 1
                for rep in range(repeat):
                    for nt in range(nt_count):
                        lhsT = xTs[:kaug, nt * P:(nt + 1) * P]
                        t2 = nt % 2
                        if t2 == 0:
                            spairF = wpool.tile([P, 2 * n], bf16, tag="sp",
                                                name="spairF", bufs=2)
                        for g in range(ngroups):
                            ps = ppool.tile([P, m_group], fp32,
                                            tag="ps", name="ps")
                            for k in range(mm_per_g):
                                nc.tensor.matmul(
                                    ps[:, k * mm_free:(k + 1) * mm_free],
                                    lhsT,
                                    yTs[:kaug,
                                        g * m_group + k * mm_free:
                                        g * m_group + (k + 1) * mm_free],
                                    start=True, stop=True)
                            blk = 2 * g + t2
                            nc.scalar.copy(
                                out=spairF[:, blk * m_group:
                                           (blk + 1) * m_group],
                                in_=ps)
                        if t2 == 1:
                            if nt == 1 and rep == 0:
                                nc.vector.tensor_copy(out=cacc2, in_=spairF)
                            else:
                                nc.vector.tensor_tensor(
                                    out=cacc2, in0=cacc2, in1=spairF, op=MIN)
                            # paired fold levels (flat 2D, both tiles at once)
                            upair = wpool.tile([P, n], bf16, tag="u",
                                               name="upair", bufs=2)
                            nc.vector.tensor_tensor(
                                out=upair, in0=spairF[:, :n],
                                in1=spairF[:, n:], op=MIN)
                            w = n
                            for _ in range(npair_folds - 1):
                                nc.vector.tensor_tensor(
                                    out=upair[:, :w // 2],
                                    in0=upair[:, :w // 2],
                                    in1=upair[:, w // 2:w], op=MIN)
                                w //= 2
                            # now upair[:, :w] = [t0_block | t1_block], each
                            # m_group wide; per-tile folds + reduce stack
                            for tt in range(2):
                                base = tt * (w // 2)
                                ww = w // 2
                                while ww > 1024:
                                    nc.vector.tensor_tensor(
                                        out=upair[:, base:base + ww // 2],
                                        in0=upair[:, base:base + ww // 2],
                                        in1=upair[:, base + ww // 2:
                                                  base + ww], op=MIN)
                                    ww //= 2
                                ntt = nt - 1 + tt
                                nc.vector.tensor_tensor(
                                    out=rstA[:, ntt % KRA, :],
                                    in0=upair[:, base:base + ww // 2],
                                    in1=upair[:, base + ww // 2:base + ww],
                                    op=MIN)
                            if (nt % KRA) == KRA - 1:
                                nc.vector.tensor_reduce(
                                    out=rowmin[:, nt - KRA + 1:nt + 1],
                                    in_=rstA[:, :, :],
                                    axis=mybir.AxisListType.X, op=MIN)
                # merge interleaved col accumulator into natural m order
                for g in range(ngroups):
                    nc.vector.tensor_tensor(
                        out=colaccX[:, g * m_group:(g + 1) * m_group],
                        in0=cacc2[:, 2 * g * m_group:(2 * g + 1) * m_group],
                        in1=cacc2[:, (2 * g + 1) * m_group:
                                  (2 * g + 2) * m_group],
                        op=MIN)
                final_colacc = colaccX

            if row_mode == "fold17":
                # Soft/exact hybrid. m-groups g0..g2 (6144 cols) stay exact
                # bf16 d2: per-tile direct MIN tensor_reduce for rows (2x on
                # HW) + a paired 2x TT-MIN column chain. Group g3 (2048 cols)
                # is evacuated by ScalarE as exp((ref-d2)/T) (ACT free
                # affine; exp from PSUM measured ~2 elem/cycle) with
                # accum_out producing the row softmin sums for free, and its
                # COLUMN accumulation runs entirely on the otherwise-idle
                # GPSIMD engine as a paired bf16 TT-ADD chain (softmin).
                # Host recovers d2 = ref - T*ln(sum) and combines.
                EXP = mybir.ActivationFunctionType.Exp
                ADD = mybir.AluOpType.add
                MAX = mybir.AluOpType.max
                we = n - m_group          # exact width 6144
                wsoft = m_group           # soft width 2048
                # gpsimd Add measured ~5ns/elem: cap its slice so the single
                # Pool engine stays under the target cadence; the rest of the
                # soft columns fold on DVE as a 2x MAX chain (exact-in-exp).
                wgp = min(wsoft, (WGP // P) * P)
                wdm = wsoft - wgp         # DVE MAX-chain slice of g3
                cacc_e = cpool.tile([P, 2, we], bf16, name="cacc_e")
                gacc = (cpool.tile([P, 2, wgp], bf16, name="gacc")
                        if wgp else None)
                cacc_m = (cpool.tile([P, 2, wdm], bf16, name="cacc_m")
                          if wdm else None)
                rsoft = cpool.tile([P, nt_count], fp32, name="rsoft")
                colsum_s = cpool.tile([P, wsoft // P], fp32, name="colsum_s")
                KR = 8
                fw = we // 8  # 768: tree folds 6144 -> 768 into the stack
                rst = cpool.tile([P, KR, fw], bf16, name="rst17")
                if wgp:
                    nc.vector.memset(gacc[:, :, :], 0.0)
                if not SOFT_ACCUM:
                    nc.vector.memset(rsoft[:, :], 0.0)
                for rep in range(repeat):
                    for nt in range(nt_count):
                        lhsT = xTs[:kaug, nt * P:(nt + 1) * P]
                        s2 = nt % 2
                        if s2 == 0:
                            spair = wpool.tile([P, 2, we], bf16, tag="sp",
                                               name="spair", bufs=2)
                            epair = wpool.tile(
                                [P, 2, wsoft], bf16, tag="ep", name="epair",
                                bufs=int(os.environ.get("CHAMFER_EP_BUFS",
                                                        "3")))
                        for g in range(ngroups):
                            ps = ppool.tile([P, m_group], fp32,
                                            tag="ps", name="ps")
                            for k in range(mm_per_g):
                                nc.tensor.matmul(
                                    ps[:, k * mm_free:(k + 1) * mm_free],
                                    lhsT,
                                    yTs[:kaug,
                                        g * m_group + k * mm_free:
                                        g * m_group + (k + 1) * mm_free],
                                    start=True, stop=True)
                            if g < ngroups - 1:
                                nc.scalar.copy(
                                    out=spair[:, s2,
                                              g * m_group:(g + 1) * m_group],
                                    in_=ps)
                            else:
                                # psum for this group is d2 - ref (ref folded
                                # into the y2 augmentation rows on host), so
                                # exp((ref-d2)/T) needs only the free scale.
                                nc.scalar.activation(
                                    out=epair[:, s2, :], in_=ps,
                                    func=EXP,
                                    scale=-1.0 / SOFT_T,
                                    bias=0.0,
                                    **({"accum_out": rsoft[:, nt:nt + 1]}
                                       if SOFT_ACCUM else {}))
                        # exact rows: fold tree 6144 -> 768 into a KR-stack,
                        # one batched 2x MIN reduce per KR tiles (the fold9
                        # recipe; wide per-tile reduces demote in situ)
                        sme = spair[:, s2, :]
                        u = wpool.tile([P, we // 2], bf16, tag="u",
                                       name="u", bufs=3)
                        nc.vector.tensor_tensor(
                            out=u, in0=sme[:, :we // 2],
                            in1=sme[:, we // 2:], op=MIN)
                        nc.vector.tensor_tensor(
                            out=u[:, :we // 4], in0=u[:, :we // 4],
                            in1=u[:, we // 4:we // 2], op=MIN)
                        nc.vector.tensor_tensor(
                            out=rst[:, nt % KR, :], in0=u[:, :we // 8],
                            in1=u[:, we // 8:we // 4], op=MIN)
                        if nt % KR == KR - 1:
                            nc.vector.tensor_reduce(
                                out=rowmin[:, nt - KR + 1:nt + 1],
                                in_=rst[:, :, :],
                                axis=mybir.AxisListType.X, op=MIN)
                        if s2 == 1:
                            # exact cols: paired 2x TT-MIN (12288 wide)
                            if nt == 1 and rep == 0:
                                nc.vector.tensor_copy(out=cacc_e, in_=spair)
                            else:
                                nc.vector.tensor_tensor(
                                    out=cacc_e, in0=cacc_e, in1=spair,
                                    op=MIN)
                        # soft cols: DVE 2x MAX chain on [0, wdm), gpsimd ADD
                        # chain on [wdm, wsoft) - both slot-alternating
                        if wdm:
                            if nt < 2 and rep == 0:
                                nc.vector.tensor_copy(
                                    out=cacc_m[:, s2, :],
                                    in_=epair[:, s2, :wdm])
                            else:
                                nc.vector.tensor_tensor(
                                    out=cacc_m[:, s2, :],
                                    in0=cacc_m[:, s2, :],
                                    in1=epair[:, s2, :wdm], op=MAX)
                        if wgp:
                            nc.gpsimd.tensor_tensor(
                                out=gacc[:, s2, :], in0=gacc[:, s2, :],
                                in1=epair[:, s2, wdm:], op=ADD)
                # merge pair slots
                nc.vector.tensor_tensor(
                    out=cacc_e[:, 0, :], in0=cacc_e[:, 0, :],
                    in1=cacc_e[:, 1, :], op=MIN)
                if wdm:
                    nc.vector.tensor_tensor(
                        out=cacc_m[:, 0, :], in0=cacc_m[:, 0, :],
                        in1=cacc_m[:, 1, :], op=MAX)
                if wgp:
                    nc.vector.tensor_tensor(
                        out=gacc[:, 0, :], in0=gacc[:, 0, :],
                        in1=gacc[:, 1, :], op=ADD)
                # exact col tail: PE transposes + 2x MIN reduces
                ntile_e = we // P
                tpb = max(1, min(ntile_e, (m_group * 2) // P))
                for t0 in range(0, ntile_e, tpb):
                    cnt = min(tpb, ntile_e - t0)
                    pt = ppool.tile([P, tpb, P], bf16, tag="ps", name="pt")
                    for i in range(cnt):
                        t = t0 + i
                        nc.tensor.transpose(
                            pt[:, i, :], cacc_e[:, 0, t * P:(t + 1) * P],
                            ident)
                    nc.vector.tensor_reduce(
                        out=colmin[:, t0:t0 + cnt], in_=pt[:, :cnt, :],
                        axis=mybir.AxisListType.X, op=MIN)
                # soft col tail: transposes + MAX reduce (cacc_m slice) and
                # ADD reduce (gacc slice); host applies ref - T*ln to both.
                ndm, ngp = wdm // P, wgp // P
                if wdm:
                    ptm = ppool.tile([P, ndm, P], bf16, tag="ps", name="ptm")
                    for i in range(ndm):
                        nc.tensor.transpose(
                            ptm[:, i, :], cacc_m[:, 0, i * P:(i + 1) * P],
                            ident)
                    nc.vector.tensor_reduce(
                        out=colsum_s[:, :ndm], in_=ptm[:, :, :],
                        axis=mybir.AxisListType.X, op=MAX)
                if wgp:
                    pt2 = ppool.tile([P, ngp, P], bf16, tag="ps", name="pt2")
                    for i in range(ngp):
                        nc.tensor.transpose(
                            pt2[:, i, :], gacc[:, 0, i * P:(i + 1) * P],
                            ident)
                    nc.vector.tensor_reduce(
                        out=colsum_s[:, ndm:], in_=pt2[:, :, :],
                        axis=mybir.AxisListType.X, op=ADD)
                nc.sync.dma_start(out[:, 0:nt_count], rowmin[:, :])
                nc.sync.dma_start(
                    out[:, nt_count:2 * nt_count], rsoft[:, :])
                nc.sync.dma_start(
                    out[:, 2 * nt_count:2 * nt_count + ntile_e],
                    colmin[:, :ntile_e])
                nc.sync.dma_start(
                    out[:, 2 * nt_count + ntile_e:
                        2 * nt_count + ntile_e + wsoft // P],
                    colsum_s[:, :])
                final_colacc = None

            if row_mode == "fold12":
                # fold9's paired column chain + the row side as ONE direct
                # tensor_reduce per tile: bf16 MIN tensor_reduce measured at
                # 2 elem/cycle on HW (same rate as the fold tree) but in a
                # single instruction - saves ~4 instruction overheads and the
                # batched-stack bookkeeping.
                cacc2 = cpool.tile([P, 2, n], bf16)
                for rep in range(repeat):
                    for nt in range(nt_count):
                        lhsT = xTs[:kaug, nt * P:(nt + 1) * P]
                        if nt % 2 == 0:
                            spair = wpool.tile([P, 2, n], bf16, tag="sp",
                                               name="spair", bufs=2)
                        for g in range(ngroups):
                            ps = ppool.tile([P, m_group], fp32,
                                            tag="ps", name="ps")
                            for k in range(mm_per_g):
                                nc.tensor.matmul(
                                    ps[:, k * mm_free:(k + 1) * mm_free],
                                    lhsT,
                                    yTs[:kaug,
                                        g * m_group + k * mm_free:
                                        g * m_group + (k + 1) * mm_free],
                                    start=True, stop=True)
                            nc.scalar.copy(
                                out=spair[:, nt % 2,
                                          g * m_group:(g + 1) * m_group],
                                in_=ps)
                        if nt % 2 == 1:
                            if nt == 1 and rep == 0:
                                nc.vector.tensor_copy(out=cacc2, in_=spair)
                            else:
                                nc.vector.tensor_tensor(
                                    out=cacc2, in0=cacc2, in1=spair, op=MIN)
                        nc.vector.tensor_reduce(
                            out=rowmin[:, nt:nt + 1],
                            in_=spair[:, nt % 2, :],
                            axis=mybir.AxisListType.X, op=MIN)
                nc.vector.tensor_tensor(
                    out=cacc2[:, 0, :], in0=cacc2[:, 0, :],
                    in1=cacc2[:, 1, :], op=MIN)
                final_colacc = cacc2[:, 0, :]

            if row_mode == "fold11":
                # fold9's paired column chain + the row side collapsed into
                # ONE DVE instruction per tile: tensor_scalar with a second
                # (accum) output reduces its own out along the free dim with
                # op1 — op0=min(x, +BIG) makes out a copy and accum_out the
                # exact row min. Runs in 4x_2p (single tensor src, bf16,
                # SBUF→SBUF) = 8192/4 cycles, vs the fold tree's ~3840.
                cacc2 = cpool.tile([P, 2, n], bf16)
                for rep in range(repeat):
                    for nt in range(nt_count):
                        lhsT = xTs[:kaug, nt * P:(nt + 1) * P]
                        if nt % 2 == 0:
                            spair = wpool.tile([P, 2, n], bf16, tag="sp",
                                               name="spair", bufs=2)
                        for g in range(ngroups):
                            ps = ppool.tile([P, m_group], fp32,
                                            tag="ps", name="ps")
                            for k in range(mm_per_g):
                                nc.tensor.matmul(
                                    ps[:, k * mm_free:(k + 1) * mm_free],
                                    lhsT,
                                    yTs[:kaug,
                                        g * m_group + k * mm_free:
                                        g * m_group + (k + 1) * mm_free],
                                    start=True, stop=True)
                            nc.scalar.copy(
                                out=spair[:, nt % 2,
                                          g * m_group:(g + 1) * m_group],
                                in_=ps)
                        if nt % 2 == 1:
                            if nt == 1 and rep == 0:
                                nc.vector.tensor_copy(out=cacc2, in_=spair)
                            else:
                                nc.vector.tensor_tensor(
                                    out=cacc2, in0=cacc2, in1=spair, op=MIN)
                        u = wpool.tile([P, n], bf16, tag="u",
                                       name="u", bufs=2)
                        nc.vector.tensor_scalar(
                            out=u,
                            in0=spair[:, nt % 2, :],
                            scalar1=3.0e38,
                            scalar2=None,
                            op0=MIN,
                            op1=MIN,
                            accum_out=rowmin[:, nt:nt + 1])
                nc.vector.tensor_tensor(
                    out=cacc2[:, 0, :], in0=cacc2[:, 0, :],
                    in1=cacc2[:, 1, :], op=MIN)
                final_colacc = cacc2[:, 0, :]

            if row_mode == "fold9":
                # fold4's per-tile tree + a PAIRED column chain: sfull tiles
                # land in [P, 2, n] pair-stacks and the column accumulator is
                # a [P, 2, n] tile (even tiles fold into slot 0, odd into
                # slot 1) updated by ONE flat 16K-wide 2x TT per pair; the
                # two slots merge once at the end. Strided (3D) TTs are
                # avoided everywhere - they demote to 1x on this hardware.
                KR9 = 8
                fw9 = n // 2
                while fw9 > 1024:
                    fw9 //= 2
                cacc2 = cpool.tile([P, 2, n], bf16)
                rst9 = cpool.tile([P, KR9, fw9 // 2], bf16)
                for rep in range(repeat):
                    for nt in range(nt_count):
                        lhsT = xTs[:kaug, nt * P:(nt + 1) * P]
                        if nt % 2 == 0:
                            spair = wpool.tile([P, 2, n], bf16, tag="sp",
                                               name="spair", bufs=2)
                        for g in range(ngroups):
                            ps = ppool.tile([P, m_group], fp32,
                                            tag="ps", name="ps")
                            for k in range(mm_per_g):
                                nc.tensor.matmul(
                                    ps[:, k * mm_free:(k + 1) * mm_free],
                                    lhsT,
                                    yTs[:kaug,
                                        g * m_group + k * mm_free:
                                        g * m_group + (k + 1) * mm_free],
                                    start=True, stop=True)
                            nc.scalar.copy(
                                out=spair[:, nt % 2,
                                          g * m_group:(g + 1) * m_group],
                                in_=ps)
                        if nt % 2 == 1:
                            if nt == 1 and rep == 0:
                                nc.vector.tensor_copy(out=cacc2, in_=spair)
                            else:
                                nc.vector.tensor_tensor(
                                    out=cacc2, in0=cacc2, in1=spair, op=MIN)
                        # per-tile row fold tree (2D contiguous slices only)
                        sme = spair[:, nt % 2, :]
                        u = wpool.tile([P, n // 2], bf16, tag="u",
                                       name="u", bufs=3)
                        nc.vector.tensor_tensor(
                            out=u, in0=sme[:, :n // 2],
                            in1=sme[:, n // 2:], op=MIN)
                        w = n // 2
                        while w > 1024:
                            nc.vector.tensor_tensor(
                                out=u[:, :w // 2], in0=u[:, :w // 2],
                                in1=u[:, w // 2:w], op=MIN)
                            w //= 2
                        nc.vector.tensor_tensor(
                            out=rst9[:, nt % KR9, :], in0=u[:, :w // 2],
                            in1=u[:, w // 2:w], op=MIN)
                        if nt % KR9 == KR9 - 1:
                            nc.vector.tensor_reduce(
                                out=rowmin[:, nt - KR9 + 1:nt + 1],
                                in_=rst9[:, :, :],
                                axis=mybir.AxisListType.X, op=MIN)
                # merge the two interleaved column accumulators in place and
                # expose the result as this mode's colacc for the tail
                nc.vector.tensor_tensor(
                    out=cacc2[:, 0, :], in0=cacc2[:, 0, :],
                    in1=cacc2[:, 1, :], op=MIN)
                final_colacc = cacc2[:, 0, :]

            if row_mode == "fold7":
                # fold4's tree shape with all sub-L1 levels BATCHED across 4
                # tiles: per tile only [colacc TT, L1-into-stack]; per 4 tiles
                # one batched TT per tree level (3D APs over [P, 4, w]
                # stacks); per 8 tiles one batched 1x reduce. Minimizes DVE
                # instruction count (~3.4/tile) at unchanged 2x busy work.
                KB7 = 4
                KR7 = 8
                ust = cpool.tile([P, KB7, n // 2], bf16)
                vst7 = cpool.tile([P, KB7, n // 4], bf16)
                wst7 = cpool.tile([P, KB7, n // 8], bf16)
                qst7 = cpool.tile([P, KB7, n // 16], bf16)
                rst7 = cpool.tile([P, KR7, n // 32], bf16)
                for rep in range(repeat):
                    for nt in range(nt_count):
                        lhsT = xTs[:kaug, nt * P:(nt + 1) * P]
                        sfull = wpool.tile([P, n], bf16, tag="s",
                                           name="sfull", bufs=3)
                        for g in range(ngroups):
                            ps = ppool.tile([P, m_group], fp32,
                                            tag="ps", name="ps")
                            for k in range(mm_per_g):
                                nc.tensor.matmul(
                                    ps[:, k * mm_free:(k + 1) * mm_free],
                                    lhsT,
                                    yTs[:kaug,
                                        g * m_group + k * mm_free:
                                        g * m_group + (k + 1) * mm_free],
                                    start=True, stop=True)
                            nc.scalar.copy(
                                out=sfull[:, g * m_group:(g + 1) * m_group],
                                in_=ps)
                        if nt == 0 and rep == 0:
                            nc.vector.tensor_copy(out=colacc, in_=sfull)
                        else:
                            nc.vector.tensor_tensor(
                                out=colacc, in0=colacc, in1=sfull, op=MIN)
                        nc.vector.tensor_tensor(
                            out=ust[:, nt % KB7, :], in0=sfull[:, :n // 2],
                            in1=sfull[:, n // 2:], op=MIN)
                        if nt % KB7 == KB7 - 1:
                            nc.vector.tensor_tensor(
                                out=vst7, in0=ust[:, :, :n // 4],
                                in1=ust[:, :, n // 4:], op=MIN)
                            nc.vector.tensor_tensor(
                                out=wst7, in0=vst7[:, :, :n // 8],
                                in1=vst7[:, :, n // 8:], op=MIN)
                            nc.vector.tensor_tensor(
                                out=qst7, in0=wst7[:, :, :n // 16],
                                in1=wst7[:, :, n // 16:], op=MIN)
                            r0 = (nt - KB7 + 1) % KR7
                            nc.vector.tensor_tensor(
                                out=rst7[:, r0:r0 + KB7, :],
                                in0=qst7[:, :, :n // 32],
                                in1=qst7[:, :, n // 32:], op=MIN)
                        if nt % KR7 == KR7 - 1:
                            nc.vector.tensor_reduce(
                                out=rowmin[:, nt - KR7 + 1:nt + 1],
                                in_=rst7[:, :, :],
                                axis=mybir.AxisListType.X, op=MIN)

            if row_mode == "fold5":
                # Fewest DVE ops per tile: colacc TT, tree L1 (8192->4096),
                # tree L2 into a per-tile stack slot (4096->2048), and ONE
                # batched 1x tensor_reduce per 4 tiles over the [P, 4, 2048]
                # stack producing 4 row minima at once.
                KR5 = 4
                for rep in range(repeat):
                    for nt in range(nt_count):
                        lhsT = xTs[:kaug, nt * P:(nt + 1) * P]
                        sfull = wpool.tile([P, n], bf16, tag="s",
                                           name="sfull", bufs=3)
                        for g in range(ngroups):
                            ps = ppool.tile([P, m_group], fp32,
                                            tag="ps", name="ps")
                            for k in range(mm_per_g):
                                nc.tensor.matmul(
                                    ps[:, k * mm_free:(k + 1) * mm_free],
                                    lhsT,
                                    yTs[:kaug,
                                        g * m_group + k * mm_free:
                                        g * m_group + (k + 1) * mm_free],
                                    start=True, stop=True)
                            dstg = sfull[:, g * m_group:(g + 1) * m_group]
                            if g >= ngroups - dve_copies:
                                nc.vector.tensor_copy(out=dstg, in_=ps)
                            else:
                                nc.scalar.copy(out=dstg, in_=ps)
                        if nt == 0 and rep == 0:
                            nc.vector.tensor_copy(out=colacc, in_=sfull)
                        else:
                            nc.vector.tensor_tensor(
                                out=colacc, in0=colacc, in1=sfull, op=MIN)
                        u = wpool.tile([P, n // 2], bf16, tag="u",
                                       name="u", bufs=3)
                        nc.vector.tensor_tensor(
                            out=u, in0=sfull[:, :n // 2],
                            in1=sfull[:, n // 2:], op=MIN)
                        if nt % KR5 == 0:
                            vst = wpool.tile([P, KR5, n // 4], bf16,
                                             tag="v5", name="vst", bufs=2)
                        nc.vector.tensor_tensor(
                            out=vst[:, nt % KR5, :], in0=u[:, :n // 4],
                            in1=u[:, n // 4:], op=MIN)
                        if nt % KR5 == KR5 - 1:
                            nc.vector.tensor_reduce(
                                out=rowmin[:, nt - KR5 + 1:nt + 1],
                                in_=vst[:, :, :],
                                axis=mybir.AxisListType.X, op=MIN)

            row_mode4 = row_mode == "fold4"
            if row_mode in ("fold", "fold4"):
                # One n-wide s tile per n-tile: ONE wide col-min TT, and row
                # mins via a fold tree of wide TT-mins and one small reduce.
                # "fold4": fold one level deeper (to 256) and batch the 1x
                # tensor_reduce across KR tiles via a [P, KR, 256] stack.
                fold_to = 256 if row_mode4 else 512
                KR = 8
                if row_mode4:
                    rstack = cpool.tile([P, KR, fold_to], bf16)
                for rep in range(repeat):
                    for nt in range(nt_count):
                        lhsT = xTs[:kaug, nt * P:(nt + 1) * P]
                        sfull = wpool.tile([P, n], bf16, tag="s",
                                           name="sfull", bufs=3)
                        for g in range(ngroups):
                            ps = ppool.tile([P, m_group], fp32,
                                            tag="ps", name="ps")
                            for k in range(mm_per_g):
                                nc.tensor.matmul(
                                    ps[:, k * mm_free:(k + 1) * mm_free],
                                    lhsT,
                                    yTs[:kaug,
                                        g * m_group + k * mm_free:
                                        g * m_group + (k + 1) * mm_free],
                                    start=True, stop=True)
                            dstg = sfull[:, g * m_group:(g + 1) * m_group]
                            if g >= ngroups - dve_copies:
                                nc.vector.tensor_copy(out=dstg, in_=ps)
                            else:
                                nc.scalar.copy(out=dstg, in_=ps)
                        if nt == 0 and rep == 0:
                            nc.vector.tensor_copy(out=colacc, in_=sfull)
                        else:
                            nc.vector.tensor_tensor(
                                out=colacc, in0=colacc, in1=sfull, op=MIN)
                        # row fold tree
                        u = wpool.tile([P, n // 2], bf16, tag="u",
                                       name="u", bufs=3)
                        nc.vector.tensor_tensor(
                            out=u, in0=sfull[:, :n // 2],
                            in1=sfull[:, n // 2:], op=MIN)
                        w = n // 2
                        while w > 2 * fold_to:
                            nc.vector.tensor_tensor(
                                out=u[:, :w // 2], in0=u[:, :w // 2],
                                in1=u[:, w // 2:w], op=MIN)
                            w //= 2
                        if row_mode4:
                            # last fold lands in this tile's reduce-stack slot
                            nc.vector.tensor_tensor(
                                out=rstack[:, nt % KR, :], in0=u[:, :w // 2],
                                in1=u[:, w // 2:w], op=MIN)
                            if nt % KR == KR - 1:
                                nc.vector.tensor_reduce(
                                    out=rowmin[:, nt - KR + 1:nt + 1],
                                    in_=rstack[:, :, :],
                                    axis=mybir.AxisListType.X, op=MIN)
                        else:
                            nc.vector.tensor_reduce(
                                out=rowmin[:, nt:nt + 1], in_=u[:, :w],
                                axis=mybir.AxisListType.X, op=MIN)

            for rep in range(
                    repeat
                    if row_mode not in ("fold", "fold4", "fold5", "fold7",
                                        "fold9", "fold10", "fold11", "fold12", "fold17", "fold2",
                                        "ttrh", "max8")
                    else 0):
              for nt in range(nt_count):
                lhsT = xTs[:kaug, nt * P:(nt + 1) * P]
                for g in range(ngroups):
                    ps = ppool.tile([P, m_group], fp32, tag="ps", name="ps")
                    for k in range(mm_per_g):
                        nc.tensor.matmul(
                            ps[:, k * mm_free:(k + 1) * mm_free],
                            lhsT,
                            yTs[:kaug, g * m_group + k * mm_free:
                                g * m_group + (k + 1) * mm_free],
                            start=True,
                            stop=True,
                        )
                    s = wpool.tile([P, m_group], bf16, name="s")
                    nc.scalar.copy(out=s, in_=ps)

                    # column-min accumulator (n folded into the 128 lanes)
                    csl = colacc[:, g * m_group:(g + 1) * m_group]
                    if nt == 0:
                        nc.vector.tensor_copy(out=csl, in_=s)
                    else:
                        nc.vector.tensor_tensor(out=csl, in0=csl, in1=s, op=MIN)

                    # row mins
                    if row_mode == "ttr2":
                        # like "ttr" but ping-pongs the elementwise-min
                        # accumulator to avoid in-place out/in1 aliasing
                        accs = [rowacc, rowacc2]
                        dst = accs[g % 2]
                        src = s if g == 0 else accs[1 - g % 2]
                        nc.vector.tensor_tensor_reduce(
                            out=dst,
                            in0=s,
                            in1=src,
                            scale=1.0,
                            scalar=3.0e38,
                            op0=MIN,
                            op1=MIN,
                            accum_out=rowmin[:, nt:nt + 1],
                        )
                    elif row_mode == "ttr":
                        # rowacc = min(rowacc, s) elementwise; accum_out gets
                        # min over the free dim of the updated rowacc. The
                        # last group's accum covers all m -> true row min.
                        nc.vector.tensor_tensor_reduce(
                            out=rowacc,
                            in0=s,
                            in1=(s if g == 0 else rowacc),
                            scale=1.0,
                            scalar=3.0e38,
                            op0=MIN,
                            op1=MIN,
                            accum_out=rowmin[:, nt:nt + 1],
                        )
                    else:
                        for k in range(mm_per_g):
                            ssl = s[:, k * mm_free:(k + 1) * mm_free]
                            if g == 0 and k == 0:
                                nc.vector.tensor_copy(out=rowacc_narrow, in_=ssl)
                            else:
                                nc.vector.tensor_tensor(
                                    out=rowacc_narrow, in0=rowacc_narrow,
                                    in1=ssl, op=MIN)
                        if g == ngroups - 1:
                            nc.vector.tensor_reduce(
                                out=rowmin[:, nt:nt + 1], in_=rowacc_narrow,
                                axis=mybir.AxisListType.X, op=MIN)

            # column-extreme finish: transpose each [128, 128] block of
            # colacc on PE, then reduce the (former partition) lanes on DVE.
            # (fold17 runs its own tail + DMA inside its mode block.)
            TAILOP = mybir.AluOpType.max if row_mode == "max8" else MIN
            if row_mode == "fold17":
                pass
            elif col_tail == "host":
                nc.sync.dma_start(colout[:, :], final_colacc[:, :])
            elif not skip_tail:
                # batch transposes into wide bf16 PSUM tiles so the lane-min
                # runs as a few wide DVE reduces instead of nt_count small ones
                tpb = max(1, min(nt_count, (m_group * 2) // P))
                for t0 in range(0, nt_count, tpb):
                    cnt = min(tpb, nt_count - t0)
                    pt = ppool.tile([P, tpb, P], bf16, tag="ps", name="pt")
                    for i in range(cnt):
                        t = t0 + i
                        nc.tensor.transpose(
                            pt[:, i, :], final_colacc[:, t * P:(t + 1) * P], ident)
                    nc.vector.tensor_reduce(
                        out=colmin[:, t0:t0 + cnt], in_=pt[:, :cnt, :],
                        axis=mybir.AxisListType.X, op=TAILOP)
            else:
                nc.vector.tensor_copy(out=colmin, in_=rowmin)

            if row_mode == "fold17":
                pass
            elif row_mode == "max8":
                nc.sync.dma_start(out[:, :rows_w], rowtop[:, :, :])
            else:
                nc.sync.dma_start(out[:, :rows_w], rowmin[:, :])
            if row_mode != "fold17" and col_tail != "host":
                nc.sync.dma_start(out[:, rows_w:], colmin[:, :])

    nc.finalize()  # runs the Bacc compile passes (event sems, reg alloc, ...)
    return nc


def _prep_inputs(x, y, kaug=KAUG, negate=False, soft_ref=None):
    """Build the augmented, transposed bf16 operands for each batch.

    negate=True flips every matmul contribution so psum = -d2 (for the
    Max-instruction row mode, where minima become maxima).
    For fold17, the softmin reference is folded into the y2 rows of the
    LAST m-group's columns so the device psum there is d2 - ref."""
    bf = ml_dtypes.bfloat16
    sg = -1.0 if negate else 1.0
    if soft_ref is None:
        mode = os.environ.get("CHAMFER_ROW_MODE", DEFAULT_ROW_MODE)
        soft_ref = SOFT_REF if mode == "fold17" else 0.0
    in_maps = []
    for b in range(x.shape[0]):
        xb = np.asarray(x[b], dtype=np.float32)
        yb = np.asarray(y[b], dtype=np.float32)
        n = xb.shape[0]
        x2 = np.sum(xb * xb, axis=-1)
        y2 = np.sum(yb * yb, axis=-1)
        if soft_ref:
            y2 = y2.copy()
            y2[-2048:] -= soft_ref
        x2_hi = (sg * x2).astype(bf)
        x2_lo = (sg * x2 - x2_hi.astype(np.float32)).astype(bf)
        y2_hi = (sg * y2).astype(bf)
        y2_lo = (sg * y2 - y2_hi.astype(np.float32)).astype(bf)
        ones = np.ones((1, n), dtype=bf)
        xT = np.concatenate(
            [xb.T.astype(bf), ones, ones, x2_hi[None], x2_lo[None]], axis=0)
        yT = np.concatenate(
            [(sg * -2.0 * yb).T.astype(bf), y2_hi[None], y2_lo[None], ones,
             ones],
            axis=0)
        if kaug > KAUG:
            pad = np.zeros((kaug - KAUG, n), dtype=bf)
            xT = np.concatenate([xT, pad], axis=0)
            yT = np.concatenate([yT, pad], axis=0)
        in_maps.append({
            "xT": np.ascontiguousarray(xT),
            "yT": np.ascontiguousarray(yT),
        })
    return in_maps


def _postprocess(results, n=N, row_mode="fold"):
    nt_count = n // P
    rows_w = nt_count * 8 if row_mode == "max8" else nt_count
    total = 0.0
    nb = len(results)
    if row_mode == "fold17":
        # out layout: [rowmin_e 64 | rsoft 64 | colmin_e 48 | colsum_s 16]
        we = n - 2048
        ne, ns = we // P, 2048 // P
        for b in range(nb):
            o = np.asarray(results[b]["out"], dtype=np.float64)
            rowmin_e = o[:, :nt_count].T.reshape(-1)          # exact, d2
            rsoft = o[:, nt_count:2 * nt_count].T.reshape(-1)  # sum exp
            with np.errstate(divide="ignore"):
                d_soft = SOFT_REF - SOFT_T * np.log(
                    np.maximum(rsoft, 1e-300))
            rowmin = np.minimum(rowmin_e, d_soft)
            colmin_e = o[:, 2 * nt_count:2 * nt_count + ne].T.reshape(-1)
            csum = o[:, 2 * nt_count + ne:
                     2 * nt_count + ne + ns].T.reshape(-1)
            with np.errstate(divide="ignore"):
                col_soft = SOFT_REF - SOFT_T * np.log(
                    np.maximum(csum, 1e-300))
            total += np.sqrt(np.maximum(rowmin, 0.0)).sum()
            total += np.sqrt(np.maximum(colmin_e, 0.0)).sum()
            total += np.sqrt(np.maximum(col_soft, 0.0)).sum()
        loss = total / nb / n
        return np.asarray(loss, dtype=np.float32)
    for b in range(nb):
        o = np.asarray(results[b]["out"], dtype=np.float64)
        if row_mode == "max8":
            # out[p, t*8+j] = j-th largest of -d2 for row n=t*128+p
            rowmin = -o[:, :rows_w].reshape(P, nt_count, 8)[:, :, 0]
            rowmin = rowmin.T.reshape(-1)
        else:
            rowmin = o[:, :rows_w].T.reshape(-1)   # [n], index t*128+p
        if "colout" in results[b]:
            co = np.asarray(results[b]["colout"], dtype=np.float32)
            colmin = co.min(axis=0).astype(np.float64)
        else:
            colmin = o[:, rows_w:].T.reshape(-1)
            if row_mode == "max8":
                colmin = -colmin
        total += np.sqrt(np.maximum(rowmin, 0.0)).sum()
        total += np.sqrt(np.maximum(colmin, 0.0)).sum()
    loss = total / nb / n
    return np.asarray(loss, dtype=np.float32)


def _get_runner(n_cores=B, row_mode=None):
    """Build the Bass module once and return a reusable jitted runner.

    Modeled on concourse.bass2jax.run_bass_via_pjrt's multi-core branch, but
    keeps the jitted callable so repeated invocations don't re-lower."""
    if row_mode is None:
        row_mode = os.environ.get("CHAMFER_ROW_MODE", DEFAULT_ROW_MODE)
    key = ("runner", n_cores, row_mode)
    if key in _CACHE:
        return _CACHE[key]

    import jax
    from jax.experimental.shard_map import shard_map
    from jax.sharding import Mesh, PartitionSpec
    from concourse import bass2jax, mybir

    nc = _build_nc(row_mode=row_mode,
                   col_tail=os.environ.get("CHAMFER_COL_TAIL", "device"),
                   dve_copies=int(os.environ.get("CHAMFER_DVE_COPIES", "0")))

    bass2jax.install_neuronx_cc_hook()
    assert nc.dbg_addr is None

    partition_name = (
        nc.partition_id_tensor.name if nc.partition_id_tensor else None)
    in_names, out_names, out_avals = [], [], []
    for alloc in nc.m.functions[0].allocations:
        if not isinstance(alloc, mybir.MemoryLocationSet):
            continue
        name = alloc.memorylocations[0].name
        if alloc.kind == "ExternalInput":
            if name != partition_name:
                in_names.append(name)
        elif alloc.kind == "ExternalOutput":
            out_names.append(name)
            out_avals.append(jax.core.ShapedArray(
                tuple(alloc.tensor_shape), mybir.dt.np(alloc.dtype)))
    n_params = len(in_names)
    n_outs = len(out_avals)
    all_in_names = list(in_names) + list(out_names)
    if partition_name is not None:
        all_in_names.append(partition_name)
    donate = tuple(range(n_params, n_params + n_outs))

    def _body(*args):
        operands = list(args)
        if partition_name is not None:
            operands.append(bass2jax.partition_id_tensor())
        outs = bass2jax._bass_exec_p.bind(
            *operands,
            out_avals=tuple(out_avals),
            in_names=tuple(all_in_names),
            out_names=tuple(out_names),
            lowering_input_output_aliases=(),
            sim_require_finite=True,
            sim_require_nnan=True,
            nc=nc,
        )
        return tuple(outs)

    devices = jax.devices()[:n_cores]
    mesh = Mesh(np.asarray(devices), ("core",))
    sharded = jax.jit(
        shard_map(
            _body, mesh=mesh,
            in_specs=(PartitionSpec("core"),) * (n_params + n_outs),
            out_specs=(PartitionSpec("core"),) * n_outs,
            check_rep=False,
        ),
        donate_argnums=donate,
        keep_unused=True,
    )

    def run(in_maps):
        per_core = [[np.asarray(m[nm]) for nm in in_names] for m in in_maps]
        concat_in = [
            np.concatenate([per_core[c][i] for c in range(n_cores)], axis=0)
            for i in range(n_params)
        ]
        concat_zeros = [
            np.zeros((n_cores * a.shape[0], *a.shape[1:]), a.dtype)
            for a in out_avals
        ]
        out_arrs = sharded(*concat_in, *concat_zeros)
        jax.block_until_ready(out_arrs)
        return [
            {nm: np.asarray(out_arrs[i]).reshape(
                n_cores, *out_avals[i].shape)[c]
             for i, nm in enumerate(out_names)}
            for c in range(n_cores)
        ]

    _CACHE[key] = run
    return run


def kernel(x, y):
    import time

    row_mode = os.environ.get("CHAMFER_ROW_MODE", DEFAULT_ROW_MODE)
    x = np.asarray(x)
    y = np.asarray(y)
    in_maps = _prep_inputs(x, y, negate=(row_mode == "max8"))
    run = _get_runner(n_cores=len(in_maps), row_mode=row_mode)
    # the device occasionally wedges transiently on a fresh NEFF's first
    # execution (NRT_EXEC_UNIT_UNRECOVERABLE); a retry reliably clears it
    last_err = None
    for attempt in range(4):
        try:
            results = run(in_maps)
            return _postprocess(results, row_mode=row_mode)
        except Exception as e:  # noqa: BLE001 - retry any runtime failure
            last_err = e
            time.sleep(2.0)
            try:
                import jax
                jax.clear_caches()
            except Exception:
                pass
            _CACHE.clear()  # rebuild runner; NEFF recompile is disk-cached
            run = _get_runner(n_cores=len(in_maps), row_mode=row_mode)
    raise last_err

